# revision 1
# baseline (speedup 1.0000x reference)
"""Trainium2 Bass kernel for nn_ClassAtt (dense MLP + 3-way class attention).

Model (per row of tube [B, 1536]):
  x1,x2,x3 = tube split into 3x512
  P_i   = relu(x_i @ w_i.T + b_i)            [B, 1024]
  last  = relu(concat(P1,P2,P3) @ wh.T + bh) [B, 1024]
  a_i   = rowwise_dot(last, P_i); w = softmax(a)  [B, 3]
  ctx   = sum_i w_i * P_i                    [B, 1024]
  out   = relu(concat(ctx, last) @ wd1.T + bd1) @ wd2.T + bd2  [B, 1000]

Strategy: pure data parallel over 8 NeuronCores (2048 rows each).  All
activations live in transposed [feature, row] layout so the contraction dim
is always on SBUF partitions and biases are per-partition scalars.  Weights
are host-transposed to [K, F].  Matmuls run as float32r (full PE rate for
free dim >= 256, ~1e-4 relative rounding) with fp32 PSUM accumulation.
Phases (DRAM spills between them):
  P1: L1 (3x 512->1024) -> PT spill   [whT chunk-loads interleaved into P1]
  P2: L2 (3072->1024) + attention -> dec spill (= [ctx; last])
  F0/F1: decode split by contraction half: out_h = relu(dec @ wd1_h.T
        + bd1_h) @ wd2_h.T; host sums the two halves (+bd2 in half 1).
DMA ordering matters: weight tensors are loaded with per-chunk DMAs
interleaved after the activation loads they must not delay (HWDGE is FIFO
per issuing engine).  The attention elementwise products run on GpSimd to
keep VectorE off the critical path.
"""

import numpy as np

import concourse.bass as bass
import concourse.mybir as mybir
import concourse.tile as tile
from concourse import bacc
from concourse.bass_utils import run_bass_kernel_spmd

F32 = mybir.dt.float32
F32R = mybir.dt.float32r

N_CORES = 8
B = 16384
ROWS = B // N_CORES  # rows per core
M = 1024             # hidden width
DEC_H = 2048
OUT = 1000

AluOp = mybir.AluOpType
Act = mybir.ActivationFunctionType


def build_nc(mm_dtype=F32R):
    nc = bacc.Bacc(None, target_bir_lowering=False)

    # ---- DRAM I/O (per-core shapes) ----
    xT = nc.dram_tensor("xT", [12, 128, ROWS], mm_dtype, kind="ExternalInput")
    wT = [
        nc.dram_tensor(f"w{i + 1}T", [4, 128, M], mm_dtype, kind="ExternalInput")
        for i in range(3)
    ]
    whT = nc.dram_tensor("whT", [24, 128, M], mm_dtype, kind="ExternalInput")
    wd1T = nc.dram_tensor("wd1T", [16, 128, DEC_H], mm_dtype, kind="ExternalInput")
    wd2T = nc.dram_tensor("wd2T", [16, 128, OUT], mm_dtype, kind="ExternalInput")
    bv = [
        nc.dram_tensor(f"b{i + 1}", [128, 8], F32, kind="ExternalInput")
        for i in range(3)
    ]
    bh = nc.dram_tensor("bh", [128, 8], F32, kind="ExternalInput")
    bd1 = nc.dram_tensor("bd1", [128, 16], F32, kind="ExternalInput")
    bd2 = nc.dram_tensor("bd2", [128, 8], F32, kind="ExternalInput")
    outH = [
        nc.dram_tensor(f"out{h}", [OUT, ROWS], F32, kind="ExternalOutput")
        for h in range(2)
    ]

    with tile.TileContext(nc) as tc:
        with tc.tile_pool(name="dram", bufs=1, space="DRAM") as dram:
            PT = dram.tile([3, 8, 128, ROWS], mm_dtype)  # P_i transposed
            dec = dram.tile([8, 128, ROWS], mm_dtype)    # last, transposed
            WS = dram.tile([128, 3, ROWS], mm_dtype)     # softmax weights

            # p2w outlives phase 1 so whT streams in during P1's compute.
            with (
                tc.tile_pool(name="p2w", bufs=1) as p2w,
                tc.tile_pool(name="psA", bufs=3, space="PSUM") as psA,
            ):
                wh_sb = p2w.tile([128, 24, M], mm_dtype)
                bh_sb = p2w.tile([128, 8], F32, tag="bh")
                ones_f32 = p2w.tile([128, 128], F32, tag="ones_f32")
                ones_sb = p2w.tile([128, 128], mm_dtype, tag="ones")

                # ------------ Phase 1: P_i = relu(x_i @ w_i.T + b_i) --------
                R1 = 256
                NT1 = ROWS // R1
                with (
                    tc.tile_pool(name="p1w", bufs=1) as p1w,
                    tc.tile_pool(name="p1x", bufs=2) as p1x,
                    tc.tile_pool(name="p1e", bufs=3) as p1e,
                ):
                    # xt tiles created lazily, loads interleaved with weights
                    xts = {}

                    def load_xt(rt):
                        rs = slice(rt * R1, (rt + 1) * R1)
                        t = p1x.tile([128, 12, R1], mm_dtype, tag="xt",
                                     name="xt")
                        for i in range(3):
                            nc.sync.dma_start(
                                t[:, 4 * i:4 * i + 4, :],
                                xT.ap()[4 * i:4 * i + 4, :, rs]
                                .rearrange("c p r -> p c r"),
                            )
                        xts[rt] = t

                    w_sb = []
                    b_sb = []
                    for i in range(3):
                        w = p1w.tile([128, 4, M], mm_dtype, tag=f"w{i}",
                                     name=f"w{i}")
                        nc.scalar.dma_start(w, wT[i].ap().rearrange("c p f -> p c f"))
                        b = p1w.tile([128, 8], F32, tag=f"b{i}", name=f"b{i}")
                        nc.scalar.dma_start(b, bv[i].ap())
                        w_sb.append(w)
                        b_sb.append(b)
                        if i < 2:
                            load_xt(i)  # first row-tiles right behind w1
                    nc.scalar.dma_start(bh_sb, bh.ap())
                    nc.any.memset(ones_f32, 1.0)
                    nc.vector.tensor_copy(ones_sb, ones_f32)

                    for rt in range(NT1):
                        # stream 3 whT chunks per row-tile behind xt prefetch
                        if rt + 2 < NT1:
                            load_xt(rt + 2)
                        for c in range(3 * rt, 3 * rt + 3):
                            nc.scalar.dma_start(wh_sb[:, c, :], whT.ap()[c])
                        rs = slice(rt * R1, (rt + 1) * R1)
                        xt = xts.pop(rt)
                        for i in range(3):
                            ev = p1e.tile([128, 8, R1], mm_dtype)
                            for fc in range(8):
                                ps = psA.tile([128, R1], F32, tag="mm",
                                              name="ps1")
                                for kc in range(4):
                                    nc.tensor.matmul(
                                        ps,
                                        w_sb[i][:, kc, fc * 128:(fc + 1) * 128],
                                        xt[:, i * 4 + kc, :],
                                        start=(kc == 0),
                                        stop=(kc == 3),
                                    )
                                nc.vector.tensor_scalar(
                                    ev[:, fc, :], ps, b_sb[i][:, fc:fc + 1],
                                    0.0, AluOp.add, AluOp.max,
                                )
                            nc.sync.dma_start(
                                PT[i, :, :, rs].rearrange("c p r -> p c r"), ev
                            )

                # ------ Phase 2: last = relu(hid1 @ wh.T + bh); attention ---
                R2 = 256
                with (
                    tc.tile_pool(name="p2pt", bufs=2) as p2pt,
                    tc.tile_pool(name="p2last", bufs=2) as p2last,
                    tc.tile_pool(name="p2big", bufs=1) as p2big,
                    tc.tile_pool(name="p2sm", bufs=1) as p2sm,
                    tc.tile_pool(name="psC", bufs=5, space="PSUM") as psC,
                ):
                    for rt in range(ROWS // R2):
                        rs = slice(rt * R2, (rt + 1) * R2)
                        pt = []
                        for i in range(3):
                            pt_i = p2pt.tile([128, 8, R2], mm_dtype,
                                             tag=f"pt{i}", name=f"pt{i}")
                            nc.sync.dma_start(
                                pt_i, PT[i, :, :, rs].rearrange("c p r -> p c r")
                            )
                            pt.append(pt_i)
                        last = p2last.tile([128, 8, R2], mm_dtype)
                        for fc in range(8):
                            ps = psA.tile([128, R2], F32, tag="mm", name="ps2")
                            for i in range(3):
                                for kc in range(8):
                                    nc.tensor.matmul(
                                        ps,
                                        wh_sb[:, i * 8 + kc,
                                              fc * 128:(fc + 1) * 128],
                                        pt[i][:, kc, :],
                                        start=(i == 0 and kc == 0),
                                        stop=(i == 2 and kc == 7),
                                    )
                            nc.scalar.activation(
                                last[:, fc, :], ps, Act.Relu,
                                bias=bh_sb[:, fc:fc + 1],
                            )
                        nc.sync.dma_start(
                            dec[:, :, rs].rearrange("c p r -> p c r"), last
                        )

                        # alphas: partition-sum of last*P_i via ones-matmul
                        # (partition-redundant [128, R2])
                        aps = []
                        for i in range(3):
                            tmp = p2big.tile([128, 8, R2], mm_dtype,
                                             tag="tmp", name=f"tmp{i}",
                                             bufs=2)
                            eng = nc.gpsimd if i == 2 else nc.vector
                            eng.tensor_tensor(tmp, last, pt[i], AluOp.mult)
                            ap_i = psC.tile([128, R2], F32, tag="alpha",
                                            name=f"alpha{i}")
                            for fc in range(8):
                                nc.tensor.matmul(
                                    ap_i, ones_sb, tmp[:, fc, :],
                                    start=(fc == 0), stop=(fc == 7),
                                )
                            aps.append(ap_i)

                        # batched softmax over the 3 logits -> WS spill
                        asb = p2sm.tile([128, 3, R2], F32, tag="asb")
                        for i in range(3):
                            nc.scalar.copy(asb[:, i, :], aps[i])
                        ai = asb.rearrange("p i r -> p r i")
                        mx = p2sm.tile([128, R2], F32, tag="mx")
                        nc.vector.reduce_max(mx, ai, axis=mybir.AxisListType.X)
                        bshp = (128, 3, R2)
                        nc.vector.tensor_tensor(
                            asb, asb, mx[:, None, :].to_broadcast(bshp),
                            AluOp.subtract)
                        nc.scalar.activation(asb, asb, Act.Exp)
                        ssum = p2sm.tile([128, R2], F32, tag="ssum")
                        nc.vector.reduce_sum(ssum, ai, axis=mybir.AxisListType.X)
                        rcp = p2sm.tile([128, R2], F32, tag="rcp")
                        nc.vector.reciprocal(rcp, ssum)
                        wsr = p2sm.tile([128, 3, R2], mm_dtype, tag="wsr")
                        nc.vector.tensor_tensor(
                            wsr, asb, rcp[:, None, :].to_broadcast(bshp),
                            AluOp.mult)
                        nc.sync.dma_start(WS[:, :, rs], wsr)

            # ---- Decode: out_h = relu(dec @ wd1_h.T + bd1_h) @ wd2_h.T -----
            RF = 256
            NTF = ROWS // RF
            for h in range(2):
                with (
                    tc.tile_pool(name=f"fw{h}", bufs=1) as fw,
                    tc.tile_pool(name=f"fd{h}", bufs=3) as fd,
                    tc.tile_pool(name=f"fo{h}", bufs=2) as fo,
                    tc.tile_pool(name=f"fe{h}", bufs=2) as fe,
                    tc.tile_pool(name=f"psF{h}", bufs=4, space="PSUM") as psF,
                    tc.tile_pool(name=f"psG{h}", bufs=4, space="PSUM") as psG,
                ):
                    dcs = {}

                    def load_dc(rt, fd=fd):
                        rs = slice(rt * RF, (rt + 1) * RF)
                        t = fd.tile([128, 16, RF], mm_dtype, tag="dc",
                                    name="dc", bufs=2)
                        nc.sync.dma_start(
                            t[:, 8:16, :], dec[:, :, rs].rearrange("c p r -> p c r")
                        )
                        wf = fd.tile([128, 3, RF], mm_dtype, tag="wf",
                                     name="wf", bufs=2)
                        nc.sync.dma_start(wf, WS[:, :, rs])
                        dcs[rt] = (t, wf)

                    wd1_sb = fw.tile([128, 16, M], mm_dtype, tag="wd1")
                    wd2_sb = fw.tile([128, 8, OUT], mm_dtype, tag="wd2")
                    bd1_sb = fw.tile([128, 8], F32, tag="bd1")
                    bd2_sb = fw.tile([128, 8], F32, tag="bd2")
                    # per-chunk weight DMAs so the first matmuls start early
                    for kc in range(16):
                        nc.scalar.dma_start(
                            wd1_sb[:, kc, :],
                            wd1T.ap()[kc, :, h * M:(h + 1) * M],
                        )
                        if kc == 0:
                            load_dc(0)
                    for kc in range(8):
                        nc.scalar.dma_start(wd2_sb[:, kc, :],
                                            wd2T.ap()[h * 8 + kc])
                    nc.scalar.dma_start(bd1_sb, bd1.ap()[:, h * 8:(h + 1) * 8])
                    if h == 1:
                        nc.scalar.dma_start(bd2_sb, bd2.ap())

                    for rt in range(NTF):
                        rs = slice(rt * RF, (rt + 1) * RF)
                        if rt + 1 < NTF:
                            load_dc(rt + 1)
                        dc, wf = dcs.pop(rt)
                        pf = fd.tile([128, 24, RF], mm_dtype, tag="ptf",
                                     name="ptf", bufs=1)
                        nc.sync.dma_start(
                            pf, PT.rearrange("i c p r -> (i c) p r")[:, :, rs]
                            .rearrange("c p r -> p c r")
                        )
                        # ctx = sum_i ws_i * P_i, written into dc[:, 0:8]
                        shp = (128, 8, RF)
                        t2 = fo.tile([128, 8, RF], F32, tag="t2", name="t2")
                        t3 = fo.tile([128, 8, RF], F32, tag="t3", name="t3")
                        nc.vector.tensor_tensor(
                            dc[:, 0:8, :],
                            wf[:, 0, None, :].to_broadcast(shp),
                            pf[:, 0:8, :], AluOp.mult)
                        nc.vector.tensor_tensor(
                            t2, wf[:, 1, None, :].to_broadcast(shp),
                            pf[:, 8:16, :], AluOp.mult)
                        nc.gpsimd.tensor_tensor(
                            t3, wf[:, 2, None, :].to_broadcast(shp),
                            pf[:, 16:24, :], AluOp.mult)
                        nc.vector.tensor_tensor(
                            dc[:, 0:8, :], dc[:, 0:8, :], t2, AluOp.add)
                        nc.vector.tensor_tensor(
                            dc[:, 0:8, :], dc[:, 0:8, :], t3, AluOp.add)
                        o1 = fo.tile([128, 8, RF], mm_dtype)
                        for fc in range(8):
                            ps = psF.tile([128, RF], F32, tag="f1")
                            for kc in range(16):
                                nc.tensor.matmul(
                                    ps,
                                    wd1_sb[:, kc, fc * 128:(fc + 1) * 128],
                                    dc[:, kc, :],
                                    start=(kc == 0),
                                    stop=(kc == 15),
                                )
                            nc.scalar.activation(
                                o1[:, fc, :], ps, Act.Relu,
                                bias=bd1_sb[:, fc:fc + 1],
                            )
                        for oc in range(8):
                            ow = 128 if oc < 7 else OUT - 7 * 128
                            ps = psG.tile([128, RF], F32, tag="f2")
                            for kc in range(8):
                                nc.tensor.matmul(
                                    ps[:ow],
                                    wd2_sb[:, kc, oc * 128:oc * 128 + ow],
                                    o1[:, kc, :],
                                    start=(kc == 0),
                                    stop=(kc == 7),
                                )
                            ev = fe.tile([128, RF], F32)
                            if h == 1:
                                nc.vector.tensor_scalar_add(
                                    ev[:ow], ps[:ow], bd2_sb[:ow, oc:oc + 1]
                                )
                            else:
                                nc.vector.tensor_copy(ev[:ow], ps[:ow])
                            nc.sync.dma_start(
                                outH[h].ap()[oc * 128:oc * 128 + ow, rs],
                                ev[:ow],
                            )

    nc.finalize()
    return nc


def _prep_inputs(tube, w1_W, w1_b, w2_W, w2_b, w3_W, w3_b, wh_W, wh_b,
                 wd1_W, wd1_b, wd2_W, wd2_b):
    """Host-side reshape/transpose into the kernel's DRAM layouts."""
    f32 = np.float32

    def wT(w, kc):  # [F, K] -> [K, F] -> [kc, 128, F]
        w = np.asarray(w, f32)
        return np.ascontiguousarray(w.T).reshape(kc, 128, w.shape[0])

    def bmat(b, cc):  # [F] -> [128, cc]
        b = np.asarray(b, f32)
        if b.shape[0] < cc * 128:
            b = np.pad(b, (0, cc * 128 - b.shape[0]))
        return np.ascontiguousarray(b.reshape(cc, 128).T)

    shared = {
        "w1T": wT(w1_W, 4), "w2T": wT(w2_W, 4), "w3T": wT(w3_W, 4),
        "whT": wT(wh_W, 24), "wd1T": wT(wd1_W, 16), "wd2T": wT(wd2_W, 16),
        "b1": bmat(w1_b, 8), "b2": bmat(w2_b, 8), "b3": bmat(w3_b, 8),
        "bh": bmat(wh_b, 8), "bd1": bmat(wd1_b, 16), "bd2": bmat(wd2_b, 8),
    }
    tubeT = np.ascontiguousarray(np.asarray(tube, f32).T)  # [1536, B]
    in_maps = []
    for c in range(N_CORES):
        xTc = np.ascontiguousarray(
            tubeT[:, c * ROWS:(c + 1) * ROWS]
        ).reshape(12, 128, ROWS)
        in_maps.append({"xT": xTc, **shared})
    return in_maps


_NC_CACHE = {}


def run(inputs, mm_dtype=F32R, trace=False):
    key = (mm_dtype, )
    if key not in _NC_CACHE:
        _NC_CACHE[key] = build_nc(mm_dtype)
    nc = _NC_CACHE[key]
    in_maps = _prep_inputs(**inputs)
    res = run_bass_kernel_spmd(nc, in_maps, list(range(N_CORES)), trace=trace)
    out = np.empty((B, OUT), np.float32)
    for c in range(N_CORES):
        r = res.results[c]
        out[c * ROWS:(c + 1) * ROWS] = (r["out0"] + r["out1"]).T
    return out, res


def kernel(**inputs) -> np.ndarray:
    out, _ = run(inputs)
    return out



# revision 2
# speedup vs baseline: 1.2905x; 1.2905x over previous
"""Trainium2 Bass kernel for nn_ClassAtt (dense MLP + 3-way class attention).

Model (per row of tube [B, 1536]):
  x1,x2,x3 = tube split into 3x512
  P_i   = relu(x_i @ w_i.T + b_i)            [B, 1024]
  last  = relu(concat(P1,P2,P3) @ wh.T + bh) [B, 1024]
  a_i   = rowwise_dot(last, P_i); w = softmax(a)  [B, 3]
  ctx   = sum_i w_i * P_i                    [B, 1024]
  out   = relu(concat(ctx, last) @ wd1.T + bd1) @ wd2.T + bd2  [B, 1000]

Strategy: pure data parallel over 8 NeuronCores (2048 rows each).  All
activations live in transposed [feature, row] layout so the contraction dim
is always on SBUF partitions and biases are per-partition scalars.  Matmuls
run in bf16 (rel-err budget is 2e-2; bf16 lands ~1e-3) with fp32 PSUM
accumulation, which halves DMA/SBUF traffic vs f32r at the same PE rate
(1 cycle/row).  All weights are SBUF-resident, packed host-side as
[fc, 128p, kc, 128f] so each output-column chunk is one contiguous DMA and
the first matmul group can start as soon as its column arrives.

Two fused passes over four 512-row chunks, one DRAM spill between them:
  Pass A: L1 (3x 512->1024), L2 (3072->1024), attention + context, all in
          SBUF; spill dec = [ctx; last] (bf16, [16,512] per row chunk).
  Pass B: out = relu(dec @ wd1.T + bd1) @ wd2.T + bd2 with the full 2048
          contraction on-chip (no host summing of halves).
Attention's partition reduction uses a ones[128,128] matmul (output is
partition-redundant, which doubles as the broadcast for ctx).  Emission is
software-pipelined: chunk k's attention matmuls are emitted between L1 and
L2 of chunk k+1 so the tensor engine never waits on the DVE products.
"""

import numpy as np
import ml_dtypes

import concourse.bass as bass
import concourse.mybir as mybir
import concourse.tile as tile
from concourse import bacc
from concourse.bass_utils import run_bass_kernel_spmd

F32 = mybir.dt.float32
F32R = mybir.dt.float32r
BF16 = mybir.dt.bfloat16

N_CORES = 8
B = 16384
ROWS = B // N_CORES   # rows per core
RT = 512              # rows per chunk
NCH = ROWS // RT      # chunks per core
M = 1024              # hidden width
DEC_H = 2048
OUT = 1000

AluOp = mybir.AluOpType
Act = mybir.ActivationFunctionType


def build_nc(mm_dtype=BF16):
    nc = bacc.Bacc(None, target_bir_lowering=False)

    # ---- DRAM I/O (per-core shapes) ----
    # x: [chunk][partition, kchunk, row]  (contiguous per partition)
    xT = nc.dram_tensor("xT", [NCH, 128, 12, RT], mm_dtype, kind="ExternalInput")
    # weights: [fc, 128p, kc, 128f] — one contiguous chunk per output column
    wv = [
        nc.dram_tensor(f"w{i + 1}", [8, 128, 4, 128], mm_dtype, kind="ExternalInput")
        for i in range(3)
    ]
    wh = nc.dram_tensor("wh", [8, 128, 24, 128], mm_dtype, kind="ExternalInput")
    wd1 = nc.dram_tensor("wd1", [16, 128, 16, 128], mm_dtype, kind="ExternalInput")
    wd2 = nc.dram_tensor("wd2", [8, 128, 16, 128], mm_dtype, kind="ExternalInput")
    bv = [
        nc.dram_tensor(f"b{i + 1}", [128, 8], F32, kind="ExternalInput")
        for i in range(3)
    ]
    bh = nc.dram_tensor("bh", [128, 8], F32, kind="ExternalInput")
    bd1 = nc.dram_tensor("bd1", [128, 16], F32, kind="ExternalInput")
    bd2 = nc.dram_tensor("bd2", [128, 8], F32, kind="ExternalInput")
    outD = nc.dram_tensor("out", [OUT, ROWS], F32, kind="ExternalOutput")

    with tile.TileContext(nc) as tc:
        with tc.tile_pool(name="dram", bufs=1, space="DRAM") as dram:
            dec = dram.tile([NCH, 128, 16, RT], mm_dtype)  # [ctx(0:8); last(8:16)]

            # ================= PASS A =================
            with (
                tc.tile_pool(name="pwA", bufs=1) as pwA,
                tc.tile_pool(name="pxt", bufs=2) as pxt,
                tc.tile_pool(name="pP", bufs=1) as pP,
                tc.tile_pool(name="plast", bufs=2) as plast,
                tc.tile_pool(name="pscr", bufs=3) as pscr,
                tc.tile_pool(name="psm", bufs=1) as psm,
                tc.tile_pool(name="pdcx", bufs=2) as pdcx,
                tc.tile_pool(name="psA", bufs=3, space="PSUM") as psA,
                tc.tile_pool(name="psC", bufs=3, space="PSUM") as psC,
            ):
                # -- resident weights / biases --
                b_sb = []
                for i in range(3):
                    b = pwA.tile([128, 8], F32, tag=f"b{i}", name=f"b{i}")
                    nc.sync.dma_start(b, bv[i].ap())
                    b_sb.append(b)
                bh_sb = pwA.tile([128, 8], F32, tag="bh")
                nc.sync.dma_start(bh_sb, bh.ap())

                w_sb = []
                for i in range(3):
                    w = pwA.tile([128, 8, 4, 128], mm_dtype, tag=f"w{i}",
                                 name=f"w{i}")
                    for fc in range(8):
                        nc.scalar.dma_start(w[:, fc], wv[i].ap()[fc])
                    w_sb.append(w)
                wh_sb = pwA.tile([128, 8, 24, 128], mm_dtype, tag="wh")
                for fc in range(8):
                    nc.scalar.dma_start(wh_sb[:, fc], wh.ap()[fc])

                ones_sb = pwA.tile([128, 128], mm_dtype, tag="ones")
                nc.any.memset(ones_sb, 1.0)

                xts = {}

                def load_xt(ch):
                    t = pxt.tile([128, 12, RT], mm_dtype, tag="xt", name="xt")
                    for i in range(3):
                        nc.sync.dma_start(t[:, 4 * i:4 * i + 4, :],
                                          xT.ap()[ch, :, 4 * i:4 * i + 4, :])
                    xts[ch] = t

                load_xt(0)
                load_xt(1)

                # per-chunk state carried into the next iteration's emission
                carry = {}

                def emit_attention(ch):
                    """alphas -> softmax -> ctx -> dec writes for chunk ch."""
                    pt, last, tmps = carry.pop(ch)
                    aps = []
                    for i in range(3):
                        ap_i = psC.tile([128, RT], F32, tag="alpha",
                                        name=f"alpha{i}")
                        for fc in range(8):
                            nc.tensor.matmul(ap_i, ones_sb, tmps[i][:, fc, :],
                                             start=(fc == 0), stop=(fc == 7))
                        aps.append(ap_i)
                    asb = psm.tile([128, 3, RT], F32, tag="asb")
                    for i in range(3):
                        nc.vector.tensor_copy(asb[:, i, :], aps[i])
                    ai = asb.rearrange("p i r -> p r i")
                    bshp = (128, 3, RT)
                    mx = psm.tile([128, RT], F32, tag="mx")
                    nc.vector.reduce_max(mx, ai, axis=mybir.AxisListType.X)
                    nc.vector.tensor_tensor(
                        asb, asb, mx[:, None, :].to_broadcast(bshp),
                        AluOp.subtract)
                    nc.scalar.activation(asb, asb, Act.Exp)
                    ssum = psm.tile([128, RT], F32, tag="ssum")
                    nc.vector.reduce_sum(ssum, ai, axis=mybir.AxisListType.X)
                    rcp = psm.tile([128, RT], F32, tag="rcp")
                    nc.vector.reciprocal(rcp, ssum)
                    wsr = psm.tile([128, 3, RT], mm_dtype, tag="wsr")
                    nc.vector.tensor_tensor(
                        wsr, asb, rcp[:, None, :].to_broadcast(bshp),
                        AluOp.mult)
                    # ctx = sum_i ws_i * P_i
                    shp = (128, 8, RT)
                    dcx = pdcx.tile([128, 8, RT], mm_dtype, tag="dcx")
                    t2 = pscr.tile([128, 8, RT], mm_dtype, tag="scr", name="t2")
                    t3 = pscr.tile([128, 8, RT], mm_dtype, tag="scr", name="t3")
                    nc.vector.tensor_tensor(
                        dcx, wsr[:, 0, None, :].to_broadcast(shp), pt[0],
                        AluOp.mult)
                    nc.vector.tensor_tensor(
                        t2, wsr[:, 1, None, :].to_broadcast(shp), pt[1],
                        AluOp.mult)
                    nc.gpsimd.tensor_tensor(
                        t3, wsr[:, 2, None, :].to_broadcast(shp), pt[2],
                        AluOp.mult)
                    nc.vector.tensor_tensor(dcx, dcx, t2, AluOp.add)
                    nc.vector.tensor_tensor(dcx, dcx, t3, AluOp.add)
                    nc.gpsimd.dma_start(dec[ch, :, 0:8, :], dcx)

                for ch in range(NCH):
                    if ch + 1 < NCH and ch > 0:
                        load_xt(ch + 1)
                    xt = xts.pop(ch)
                    # ---- L1: P_i = relu(x_i @ w_i.T + b_i) ----
                    pt = []
                    for i in range(3):
                        p_i = pP.tile([128, 8, RT], mm_dtype, tag=f"p{i}",
                                      name=f"p{i}")
                        for fc in range(8):
                            ps = psA.tile([128, RT], F32, tag="mm", name="ps1")
                            for kc in range(4):
                                nc.tensor.matmul(
                                    ps, w_sb[i][:, fc, kc, :],
                                    xt[:, 4 * i + kc, :],
                                    start=(kc == 0), stop=(kc == 3))
                            nc.scalar.activation(p_i[:, fc, :], ps, Act.Relu,
                                                 bias=b_sb[i][:, fc:fc + 1])
                        pt.append(p_i)

                    # attention of the previous chunk hides inside this L1/L2
                    if ch > 0:
                        emit_attention(ch - 1)

                    # ---- L2: last = relu(concat(P) @ wh.T + bh) ----
                    last = plast.tile([128, 8, RT], mm_dtype, tag="last")
                    for fc in range(8):
                        ps = psA.tile([128, RT], F32, tag="mm", name="ps2")
                        for i in range(3):
                            for kc in range(8):
                                nc.tensor.matmul(
                                    ps, wh_sb[:, fc, 8 * i + kc, :],
                                    pt[i][:, kc, :],
                                    start=(i == 0 and kc == 0),
                                    stop=(i == 2 and kc == 7))
                        nc.scalar.activation(last[:, fc, :], ps, Act.Relu,
                                             bias=bh_sb[:, fc:fc + 1])
                    nc.gpsimd.dma_start(dec[ch, :, 8:16, :], last)

                    # products for alphas (consumed by emit_attention(ch))
                    tmps = []
                    for i in range(3):
                        tmp = pscr.tile([128, 8, RT], mm_dtype, tag="scr",
                                        name=f"tmp{i}")
                        eng = nc.gpsimd if i == 2 else nc.vector
                        eng.tensor_tensor(tmp, last, pt[i], AluOp.mult)
                        tmps.append(tmp)
                    carry[ch] = (pt, last, tmps)

                emit_attention(NCH - 1)

            # ================= PASS B =================
            with (
                tc.tile_pool(name="pwB", bufs=1) as pwB,
                tc.tile_pool(name="pdc", bufs=2) as pdc,
                tc.tile_pool(name="po1", bufs=2) as po1,
                tc.tile_pool(name="pev", bufs=3) as pev,
                tc.tile_pool(name="psF", bufs=3, space="PSUM") as psF,
                tc.tile_pool(name="psG", bufs=3, space="PSUM") as psG,
            ):
                bd1_sb = pwB.tile([128, 16], F32, tag="bd1")
                nc.sync.dma_start(bd1_sb, bd1.ap())
                bd2_sb = pwB.tile([128, 8], F32, tag="bd2")
                nc.sync.dma_start(bd2_sb, bd2.ap())

                dcs = {}

                def load_dc(ch):
                    t = pdc.tile([128, 16, RT], mm_dtype, tag="dc", name="dc")
                    nc.sync.dma_start(t, dec[ch])
                    dcs[ch] = t

                wd1_sb = pwB.tile([128, 16, 16, 128], mm_dtype, tag="wd1")
                wd2_sb = pwB.tile([128, 8, 16, 128], mm_dtype, tag="wd2")
                load_dc(0)
                # column-major weight streams so mm1 fc=0 starts immediately
                for fc in range(16):
                    nc.scalar.dma_start(wd1_sb[:, fc], wd1.ap()[fc])
                    if fc == 1:
                        load_dc(1)
                for fc in range(8):
                    nc.scalar.dma_start(wd2_sb[:, fc], wd2.ap()[fc])

                for ch in range(NCH):
                    rs = slice(ch * RT, (ch + 1) * RT)
                    if ch + 2 < NCH:
                        load_dc(ch + 2)
                    dc = dcs.pop(ch)
                    o1 = po1.tile([128, 16, RT], mm_dtype, tag="o1")
                    for fc in range(16):
                        ps = psF.tile([128, RT], F32, tag="f1")
                        for kc in range(16):
                            nc.tensor.matmul(ps, wd1_sb[:, fc, kc, :],
                                             dc[:, kc, :],
                                             start=(kc == 0), stop=(kc == 15))
                        nc.scalar.activation(o1[:, fc, :], ps, Act.Relu,
                                             bias=bd1_sb[:, fc:fc + 1])
                    for oc in range(8):
                        ow = 128 if oc < 7 else OUT - 7 * 128
                        ps = psG.tile([128, RT], F32, tag="f2")
                        for kc in range(16):
                            nc.tensor.matmul(ps, wd2_sb[:, oc, kc, :],
                                             o1[:, kc, :],
                                             start=(kc == 0), stop=(kc == 15))
                        ev = pev.tile([128, RT], F32, tag="ev")
                        nc.vector.tensor_scalar_add(ev, ps,
                                                    bd2_sb[:, oc:oc + 1])
                        nc.gpsimd.dma_start(
                            outD.ap()[oc * 128:oc * 128 + ow, rs], ev[:ow])

    nc.finalize()
    return nc


def _wpack(W, FC, KC, np_dt):
    """[F_out, K_in] -> [FC, 128p, KC, 128f] (pad F_out up to FC*128)."""
    W = np.asarray(W, np.float32)
    F, K = W.shape
    if F < FC * 128:
        W = np.pad(W, ((0, FC * 128 - F), (0, 0)))
    W4 = W.reshape(FC, 128, KC, 128)          # [fc, f, kc, p]
    return np.ascontiguousarray(W4.transpose(0, 3, 2, 1)).astype(np_dt)


def _bmat(b, cc):
    """[F] -> [128, cc] so column c holds b[c*128:(c+1)*128]."""
    b = np.asarray(b, np.float32)
    if b.shape[0] < cc * 128:
        b = np.pad(b, (0, cc * 128 - b.shape[0]))
    return np.ascontiguousarray(b.reshape(cc, 128).T)


def _prep_inputs(np_dt, tube, w1_W, w1_b, w2_W, w2_b, w3_W, w3_b, wh_W, wh_b,
                 wd1_W, wd1_b, wd2_W, wd2_b):
    shared = {
        "w1": _wpack(w1_W, 8, 4, np_dt), "w2": _wpack(w2_W, 8, 4, np_dt),
        "w3": _wpack(w3_W, 8, 4, np_dt), "wh": _wpack(wh_W, 8, 24, np_dt),
        "wd1": _wpack(wd1_W, 16, 16, np_dt), "wd2": _wpack(wd2_W, 8, 16, np_dt),
        "b1": _bmat(w1_b, 8), "b2": _bmat(w2_b, 8), "b3": _bmat(w3_b, 8),
        "bh": _bmat(wh_b, 8), "bd1": _bmat(wd1_b, 16), "bd2": _bmat(wd2_b, 8),
    }
    tube = np.asarray(tube, np.float32)
    in_maps = []
    for c in range(N_CORES):
        t = tube[c * ROWS:(c + 1) * ROWS]                # [ROWS, 1536]
        xc = np.ascontiguousarray(
            t.reshape(NCH, RT, 12, 128).transpose(0, 3, 2, 1)
        ).astype(np_dt)                                  # [NCH, 128, 12, RT]
        in_maps.append({"xT": xc, **shared})
    return in_maps


_NC_CACHE = {}


def run(inputs, mm_dtype=BF16, trace=False):
    key = (mm_dtype,)
    if key not in _NC_CACHE:
        _NC_CACHE[key] = build_nc(mm_dtype)
    nc = _NC_CACHE[key]
    np_dt = ml_dtypes.bfloat16 if mm_dtype == BF16 else np.float32
    in_maps = _prep_inputs(np_dt, **inputs)
    res = run_bass_kernel_spmd(nc, in_maps, list(range(N_CORES)), trace=trace)
    out = np.empty((B, OUT), np.float32)
    for c in range(N_CORES):
        out[c * ROWS:(c + 1) * ROWS] = res.results[c]["out"].T
    return out, res


def kernel(**inputs) -> np.ndarray:
    out, _ = run(inputs)
    return out


# revision 4
# speedup vs baseline: 1.5099x; 1.1700x over previous
"""Trainium2 Bass kernel for nn_ClassAtt (dense MLP + 3-way class attention).

Model (per row of tube [B, 1536]):
  x1,x2,x3 = tube split into 3x512
  P_i   = relu(x_i @ w_i.T + b_i)            [B, 1024]
  last  = relu(concat(P1,P2,P3) @ wh.T + bh) [B, 1024]
  a_i   = rowwise_dot(last, P_i); w = softmax(a)  [B, 3]
  ctx   = sum_i w_i * P_i                    [B, 1024]
  out   = relu(concat(ctx, last) @ wd1.T + bd1) @ wd2.T + bd2  [B, 1000]

Strategy: pure data parallel over 8 NeuronCores (2048 rows each).  All
activations live in transposed [feature, row] layout so the contraction dim
is always on SBUF partitions and biases are per-partition scalars.  Matmuls
run in bf16 (rel-err budget is 2e-2; bf16 lands ~6e-3) with fp32 PSUM
accumulation, which halves DMA/SBUF traffic vs f32r at the same PE rate
(1 cycle/row).  All weights are SBUF-resident, packed host-side as
[fc, 128p, kc, 128f] so each output-column chunk is one contiguous DMA and
a matmul group can start as soon as its column arrives.

Two fused passes, one DRAM spill between them:
  Pass A (8 chunks of 256 rows): L1, L2, attention + context in SBUF;
          spill dec = [ctx; last] (bf16).
  Pass B (4 chunks of 512 rows): out = relu(dec @ wd1.T + bd1) @ wd2.T
          + bd2 with the full 2048 contraction on-chip.
Attention's partition reduction uses a ones[128,128] matmul (output is
partition-redundant, which doubles as the broadcast for ctx).  All
attention elementwise ops are contiguous 2D [128, R] — broadcast APs and
large GpSimd ops measured 4x slower on DVE/GpSimd and are avoided.
Softmax runs in PSUM (exp on ScalarE).  Emission is software-pipelined:
chunk k's attention matmuls are emitted between L1 and L2 of chunk k+1.
Pass-B weights stream on the sync DMA queue (idle during pass A tail) and
dec row-chunks prefetch into an outer-scope pool so the pass transition
costs ~no tensor idle.
"""

import numpy as np
import ml_dtypes

import concourse.bass as bass
import concourse.mybir as mybir
import concourse.tile as tile
from concourse import bacc
from concourse.bass_utils import run_bass_kernel_spmd

F32 = mybir.dt.float32
F32R = mybir.dt.float32r
BF16 = mybir.dt.bfloat16

N_CORES = 8
B = 16384
ROWS = B // N_CORES   # rows per core
RTA = 256             # pass-A rows per chunk
NCHA = ROWS // RTA
RTB = 512             # pass-B rows per chunk
NCHB = ROWS // RTB
M = 1024              # hidden width
DEC_H = 2048
OUT = 1000

AluOp = mybir.AluOpType
Act = mybir.ActivationFunctionType


def build_nc(mm_dtype=BF16):
    nc = bacc.Bacc(None, target_bir_lowering=False)

    # ---- DRAM I/O (per-core shapes) ----
    # x: [chunk][partition, kchunk, row]  (contiguous per partition)
    xT = nc.dram_tensor("xT", [NCHA, 128, 12, RTA], mm_dtype,
                        kind="ExternalInput")
    # weights: [fc, 128p, kc, 128f] — one contiguous chunk per output column
    wv = [
        nc.dram_tensor(f"w{i + 1}", [8, 128, 4, 128], mm_dtype,
                       kind="ExternalInput")
        for i in range(3)
    ]
    wh = nc.dram_tensor("wh", [8, 128, 24, 128], mm_dtype, kind="ExternalInput")
    wd1 = nc.dram_tensor("wd1", [16, 128, 16, 128], mm_dtype,
                         kind="ExternalInput")
    wd2 = nc.dram_tensor("wd2", [8, 128, 16, 128], mm_dtype,
                         kind="ExternalInput")
    bv = [
        nc.dram_tensor(f"b{i + 1}", [128, 8], F32, kind="ExternalInput")
        for i in range(3)
    ]
    bh = nc.dram_tensor("bh", [128, 8], F32, kind="ExternalInput")
    bd1 = nc.dram_tensor("bd1", [128, 16], F32, kind="ExternalInput")
    bd2 = nc.dram_tensor("bd2", [128, 8], F32, kind="ExternalInput")
    outD = nc.dram_tensor("out", [OUT, ROWS], F32, kind="ExternalOutput")

    with tile.TileContext(nc) as tc:
        with tc.tile_pool(name="dram", bufs=1, space="DRAM") as dram:
            # dec indexed by pass-B chunk; pass-A chunks write half each
            dec = dram.tile([NCHB, 128, 16, RTB], mm_dtype)

            # outer-scope pool: dec prefetch survives the pass transition
            with tc.tile_pool(name="pdc", bufs=2) as pdc:
                dcs = {}

                def load_dc(chb):
                    t = pdc.tile([128, 16, RTB], mm_dtype, tag="dc", name="dc")
                    nc.sync.dma_start(t, dec[chb])
                    dcs[chb] = t

                # ================= PASS A =================
                with (
                    tc.tile_pool(name="pwA", bufs=1) as pwA,
                    tc.tile_pool(name="pxt", bufs=3) as pxt,
                    tc.tile_pool(name="pP", bufs=2) as pP,
                    tc.tile_pool(name="plast", bufs=1) as plast,
                    tc.tile_pool(name="pscr", bufs=3) as pscr,
                    tc.tile_pool(name="psm", bufs=1) as psm,
                    tc.tile_pool(name="pdcx", bufs=1) as pdcx,
                    tc.tile_pool(name="psA", bufs=3, space="PSUM") as psA,
                    tc.tile_pool(name="psC", bufs=3, space="PSUM") as psC,
                ):
                    # -- resident weights / biases --
                    b_sb = []
                    for i in range(3):
                        b = pwA.tile([128, 8], F32, tag=f"b{i}", name=f"b{i}")
                        nc.sync.dma_start(b, bv[i].ap())
                        b_sb.append(b)
                    bh_sb = pwA.tile([128, 8], F32, tag="bh")
                    nc.sync.dma_start(bh_sb, bh.ap())

                    w_sb = []
                    for i in range(3):
                        w = pwA.tile([128, 8, 4, 128], mm_dtype, tag=f"w{i}",
                                     name=f"w{i}")
                        for fc in range(8):
                            nc.scalar.dma_start(w[:, fc], wv[i].ap()[fc])
                        w_sb.append(w)
                    wh_sb = pwA.tile([128, 8, 24, 128], mm_dtype, tag="wh")
                    for fc in range(8):
                        nc.scalar.dma_start(wh_sb[:, fc], wh.ap()[fc])

                    ones_sb = pwA.tile([128, 128], mm_dtype, tag="ones")
                    nc.any.memset(ones_sb, 1.0)

                    xts = {}

                    def load_xt(ch, split=False):
                        t = pxt.tile([128, 12, RTA], mm_dtype, tag="xt",
                                     name="xt")
                        if split:
                            for i in range(3):
                                nc.sync.dma_start(
                                    t[:, 4 * i:4 * i + 4, :],
                                    xT.ap()[ch, :, 4 * i:4 * i + 4, :])
                        else:
                            nc.sync.dma_start(t, xT.ap()[ch])
                        xts[ch] = t

                    load_xt(0, split=True)
                    load_xt(1)
                    load_xt(2)

                    carry = {}

                    def emit_attention(ch):
                        """alphas -> softmax -> ctx -> dec writes, chunk ch."""
                        pt, last, tmps = carry.pop(ch)
                        aps = []
                        for i in range(3):
                            ap_i = psC.tile([128, RTA], F32, tag="alpha",
                                            name=f"alpha{i}")
                            for fc in range(8):
                                nc.tensor.matmul(ap_i, ones_sb,
                                                 tmps[i][:, fc, :],
                                                 start=(fc == 0),
                                                 stop=(fc == 7))
                            aps.append(ap_i)
                        # softmax over 3 logits, all contiguous 2D ops.
                        # (DVE may read at most one PSUM operand, so stage
                        # the logits into SBUF on ScalarE first.)
                        a = []
                        for i in range(3):
                            a_i = psm.tile([128, RTA], F32, tag=f"a{i}",
                                           name=f"a{i}")
                            nc.scalar.copy(a_i, aps[i])
                            a.append(a_i)
                        mx = psm.tile([128, RTA], F32, tag="mx")
                        nc.vector.tensor_tensor(mx, a[0], a[1], AluOp.max)
                        nc.vector.tensor_tensor(mx, mx, a[2], AluOp.max)
                        for i in range(3):
                            nc.vector.tensor_tensor(a[i], a[i], mx,
                                                    AluOp.subtract)
                            nc.scalar.activation(a[i], a[i], Act.Exp)
                        ssum = psm.tile([128, RTA], F32, tag="ssum")
                        nc.vector.tensor_tensor(ssum, a[0], a[1], AluOp.add)
                        nc.vector.tensor_tensor(ssum, ssum, a[2], AluOp.add)
                        rcp = psm.tile([128, RTA], F32, tag="rcp")
                        nc.vector.reciprocal(rcp, ssum)
                        ws = []
                        for i in range(3):
                            ws_i = psm.tile([128, RTA], mm_dtype,
                                            tag=f"ws{i}", name=f"ws{i}")
                            nc.vector.tensor_tensor(ws_i, a[i], rcp,
                                                    AluOp.mult)
                            ws.append(ws_i)
                        # ctx = sum_i ws_i * P_i — per-fc contiguous 2D ops
                        dcx = pdcx.tile([128, 8, RTA], mm_dtype, tag="dcx")
                        t2 = pscr.tile([128, 8, RTA], mm_dtype, tag="scr",
                                       name="t2")
                        for fc in range(8):
                            nc.vector.tensor_tensor(
                                dcx[:, fc, :], ws[0], pt[0][:, fc, :],
                                AluOp.mult)
                            nc.vector.tensor_tensor(
                                t2[:, fc, :], ws[1], pt[1][:, fc, :],
                                AluOp.mult)
                            nc.vector.tensor_tensor(
                                dcx[:, fc, :], dcx[:, fc, :], t2[:, fc, :],
                                AluOp.add)
                            nc.vector.tensor_tensor(
                                t2[:, fc, :], ws[2], pt[2][:, fc, :],
                                AluOp.mult)
                            nc.vector.tensor_tensor(
                                dcx[:, fc, :], dcx[:, fc, :], t2[:, fc, :],
                                AluOp.add)
                        rh = slice((ch % 2) * RTA, (ch % 2) * RTA + RTA)
                        nc.gpsimd.dma_start(dec[ch // 2, :, 0:8, rh], dcx)

                    for ch in range(NCHA):
                        xt = xts.pop(ch)
                        # ---- L1: P_i = relu(x_i @ w_i.T + b_i) ----
                        pt = []
                        for i in range(3):
                            p_i = pP.tile([128, 8, RTA], mm_dtype,
                                          tag=f"p{i}", name=f"p{i}")
                            for fc in range(8):
                                ps = psA.tile([128, RTA], F32, tag="mm",
                                              name="ps1")
                                for kc in range(4):
                                    nc.tensor.matmul(
                                        ps, w_sb[i][:, fc, kc, :],
                                        xt[:, 4 * i + kc, :],
                                        start=(kc == 0), stop=(kc == 3))
                                nc.scalar.activation(
                                    p_i[:, fc, :], ps, Act.Relu,
                                    bias=b_sb[i][:, fc:fc + 1])
                            pt.append(p_i)
                        # xt(ch) fully consumed; safe to recycle its buffer
                        if ch + 3 < NCHA:
                            load_xt(ch + 3)

                        # previous chunk's attention hides inside this L1/L2
                        if ch > 0:
                            emit_attention(ch - 1)

                        # ---- L2: last = relu(concat(P) @ wh.T + bh) ----
                        last = plast.tile([128, 8, RTA], mm_dtype, tag="last")
                        for fc in range(8):
                            ps = psA.tile([128, RTA], F32, tag="mm",
                                          name="ps2")
                            for i in range(3):
                                for kc in range(8):
                                    nc.tensor.matmul(
                                        ps, wh_sb[:, fc, 8 * i + kc, :],
                                        pt[i][:, kc, :],
                                        start=(i == 0 and kc == 0),
                                        stop=(i == 2 and kc == 7))
                            nc.scalar.activation(last[:, fc, :], ps, Act.Relu,
                                                 bias=bh_sb[:, fc:fc + 1])
                        rh = slice((ch % 2) * RTA, (ch % 2) * RTA + RTA)
                        nc.gpsimd.dma_start(dec[ch // 2, :, 8:16, rh], last)

                        # products for alphas (consumed by emit_attention(ch))
                        tmps = []
                        for i in range(3):
                            tmp = pscr.tile([128, 8, RTA], mm_dtype,
                                            tag="scr", name=f"tmp{i}")
                            nc.vector.tensor_tensor(tmp, last, pt[i],
                                                    AluOp.mult)
                            tmps.append(tmp)
                        carry[ch] = (pt, last, tmps)

                    emit_attention(NCHA - 1)

                # ================= PASS B =================
                with (
                    tc.tile_pool(name="pwB", bufs=1) as pwB,
                    tc.tile_pool(name="po1", bufs=2) as po1,
                    tc.tile_pool(name="pev", bufs=3) as pev,
                    tc.tile_pool(name="psF", bufs=3, space="PSUM") as psF,
                    tc.tile_pool(name="psG", bufs=3, space="PSUM") as psG,
                ):
                    bd1_sb = pwB.tile([128, 16], F32, tag="bd1")
                    nc.sync.dma_start(bd1_sb, bd1.ap())
                    bd2_sb = pwB.tile([128, 8], F32, tag="bd2")
                    nc.sync.dma_start(bd2_sb, bd2.ap())

                    load_dc(0)
                    load_dc(1)
                    # pass-B weights on the sync queue: it drains early in
                    # pass A, so these stream in as soon as their aliased
                    # SBUF (pass-A tiles) is released
                    wd1_sb = pwB.tile([128, 16, 16, 128], mm_dtype, tag="wd1")
                    wd2_sb = pwB.tile([128, 8, 16, 128], mm_dtype, tag="wd2")
                    for fc in range(16):
                        nc.sync.dma_start(wd1_sb[:, fc], wd1.ap()[fc])
                    for fc in range(8):
                        nc.sync.dma_start(wd2_sb[:, fc], wd2.ap()[fc])

                    for ch in range(NCHB):
                        rs = slice(ch * RTB, (ch + 1) * RTB)
                        dc = dcs.pop(ch)
                        o1 = po1.tile([128, 16, RTB], mm_dtype, tag="o1")
                        for fc in range(16):
                            ps = psF.tile([128, RTB], F32, tag="f1")
                            for kc in range(16):
                                nc.tensor.matmul(ps, wd1_sb[:, fc, kc, :],
                                                 dc[:, kc, :],
                                                 start=(kc == 0),
                                                 stop=(kc == 15))
                            nc.scalar.activation(o1[:, fc, :], ps, Act.Relu,
                                                 bias=bd1_sb[:, fc:fc + 1])
                        # dc(ch) fully consumed; recycle its buffer
                        if ch + 2 < NCHB:
                            load_dc(ch + 2)
                        for oc in range(8):
                            ow = 128 if oc < 7 else OUT - 7 * 128
                            ps = psG.tile([128, RTB], F32, tag="f2")
                            for kc in range(16):
                                nc.tensor.matmul(ps, wd2_sb[:, oc, kc, :],
                                                 o1[:, kc, :],
                                                 start=(kc == 0),
                                                 stop=(kc == 15))
                            ev = pev.tile([128, RTB], F32, tag="ev")
                            nc.vector.tensor_scalar_add(ev, ps,
                                                        bd2_sb[:, oc:oc + 1])
                            nc.gpsimd.dma_start(
                                outD.ap()[oc * 128:oc * 128 + ow, rs],
                                ev[:ow])

    nc.finalize()
    return nc


def _wpack(W, FC, KC, np_dt):
    """[F_out, K_in] -> [FC, 128p, KC, 128f] (pad F_out up to FC*128)."""
    W = np.asarray(W, np.float32)
    F, K = W.shape
    if F < FC * 128:
        W = np.pad(W, ((0, FC * 128 - F), (0, 0)))
    W4 = W.reshape(FC, 128, KC, 128)          # [fc, f, kc, p]
    return np.ascontiguousarray(W4.transpose(0, 3, 2, 1)).astype(np_dt)


def _bmat(b, cc):
    """[F] -> [128, cc] so column c holds b[c*128:(c+1)*128]."""
    b = np.asarray(b, np.float32)
    if b.shape[0] < cc * 128:
        b = np.pad(b, (0, cc * 128 - b.shape[0]))
    return np.ascontiguousarray(b.reshape(cc, 128).T)


def _prep_inputs(np_dt, tube, w1_W, w1_b, w2_W, w2_b, w3_W, w3_b, wh_W, wh_b,
                 wd1_W, wd1_b, wd2_W, wd2_b):
    shared = {
        "w1": _wpack(w1_W, 8, 4, np_dt), "w2": _wpack(w2_W, 8, 4, np_dt),
        "w3": _wpack(w3_W, 8, 4, np_dt), "wh": _wpack(wh_W, 8, 24, np_dt),
        "wd1": _wpack(wd1_W, 16, 16, np_dt), "wd2": _wpack(wd2_W, 8, 16, np_dt),
        "b1": _bmat(w1_b, 8), "b2": _bmat(w2_b, 8), "b3": _bmat(w3_b, 8),
        "bh": _bmat(wh_b, 8), "bd1": _bmat(wd1_b, 16), "bd2": _bmat(wd2_b, 8),
    }
    tube = np.asarray(tube, np.float32)
    in_maps = []
    for c in range(N_CORES):
        t = tube[c * ROWS:(c + 1) * ROWS]                # [ROWS, 1536]
        xc = np.ascontiguousarray(
            t.reshape(NCHA, RTA, 12, 128).transpose(0, 3, 2, 1)
        ).astype(np_dt)                                  # [NCHA, 128, 12, RTA]
        in_maps.append({"xT": xc, **shared})
    return in_maps


_NC_CACHE = {}


def run(inputs, mm_dtype=BF16, trace=False):
    key = (mm_dtype,)
    if key not in _NC_CACHE:
        _NC_CACHE[key] = build_nc(mm_dtype)
    nc = _NC_CACHE[key]
    np_dt = ml_dtypes.bfloat16 if mm_dtype == BF16 else np.float32
    in_maps = _prep_inputs(np_dt, **inputs)
    res = run_bass_kernel_spmd(nc, in_maps, list(range(N_CORES)), trace=trace)
    out = np.empty((B, OUT), np.float32)
    for c in range(N_CORES):
        out[c * ROWS:(c + 1) * ROWS] = res.results[c]["out"].T
    return out, res


def kernel(**inputs) -> np.ndarray:
    out, _ = run(inputs)
    return out


# revision 9
# speedup vs baseline: 1.5399x; 1.0199x over previous
"""Trainium2 Bass kernel for nn_ClassAtt (dense MLP + 3-way class attention).

Model (per row of tube [B, 1536]):
  x1,x2,x3 = tube split into 3x512
  P_i   = relu(x_i @ w_i.T + b_i)            [B, 1024]
  last  = relu(concat(P1,P2,P3) @ wh.T + bh) [B, 1024]
  a_i   = rowwise_dot(last, P_i); w = softmax(a)  [B, 3]
  ctx   = sum_i w_i * P_i                    [B, 1024]
  out   = relu(concat(ctx, last) @ wd1.T + bd1) @ wd2.T + bd2  [B, 1000]

Strategy: pure data parallel over 8 NeuronCores (2048 rows each).  All
activations live in transposed [feature, row] layout so the contraction dim
is always on SBUF partitions and biases are per-partition scalars.  Matmuls
run in bf16 (rel-err budget is 2e-2; bf16 lands ~6e-3) with fp32 PSUM
accumulation, which halves DMA/SBUF traffic vs f32r at the same PE rate
(1 cycle/row).  All weights are SBUF-resident, packed host-side as
[fc, 128p, kc, 128f] so each output-column chunk is one contiguous DMA and
a matmul group can start as soon as its column arrives.

Two fused passes, one DRAM spill between them:
  Pass A (8 chunks of 256 rows): L1, L2, attention + context in SBUF;
          spill dec = [ctx; last] (bf16).
  Pass B (4 chunks of 512 rows): out = relu(dec @ wd1.T + bd1) @ wd2.T
          + bd2 with the full 2048 contraction on-chip.
Attention's partition reduction uses a ones[128,128] matmul (output is
partition-redundant, which doubles as the broadcast for ctx).  All
attention elementwise ops are contiguous 2D [128, R] — broadcast APs and
large GpSimd ops measured 4x slower on DVE/GpSimd and are avoided.
Softmax runs in PSUM (exp on ScalarE).  Emission is software-pipelined:
chunk k's attention matmuls are emitted between L1 and L2 of chunk k+1.
Pass-B weights stream on the sync DMA queue (idle during pass A tail) and
dec row-chunks prefetch into an outer-scope pool so the pass transition
costs ~no tensor idle.
"""

import numpy as np
import ml_dtypes

import concourse.bass as bass
import concourse.mybir as mybir
import concourse.tile as tile
from concourse import bacc
from concourse.bass_utils import run_bass_kernel_spmd

F32 = mybir.dt.float32
F32R = mybir.dt.float32r
BF16 = mybir.dt.bfloat16

N_CORES = 8
B = 16384
ROWS = B // N_CORES   # rows per core
RTA = 256             # pass-A rows per chunk
NCHA = ROWS // RTA
RTB = 512             # pass-B rows per chunk
NCHB = ROWS // RTB
M = 1024              # hidden width
DEC_H = 2048
OUT = 1000

AluOp = mybir.AluOpType
Act = mybir.ActivationFunctionType


def build_nc(mm_dtype=BF16):
    nc = bacc.Bacc(None, target_bir_lowering=False)

    # ---- DRAM I/O (per-core shapes) ----
    # x: [chunk][partition, kchunk, row]  (contiguous per partition)
    xT = nc.dram_tensor("xT", [NCHA, 128, 12, RTA], mm_dtype,
                        kind="ExternalInput")
    # weights: [fc, 128p, kc, 128f] — one contiguous chunk per output column
    wv = [
        nc.dram_tensor(f"w{i + 1}", [8, 128, 4, 128], mm_dtype,
                       kind="ExternalInput")
        for i in range(3)
    ]
    wh = nc.dram_tensor("wh", [8, 128, 24, 128], mm_dtype, kind="ExternalInput")
    wd1 = nc.dram_tensor("wd1", [16, 128, 16, 128], mm_dtype,
                         kind="ExternalInput")
    wd2 = nc.dram_tensor("wd2", [8, 128, 16, 128], mm_dtype,
                         kind="ExternalInput")
    bv = [
        nc.dram_tensor(f"b{i + 1}", [128, 8], F32, kind="ExternalInput")
        for i in range(3)
    ]
    bh = nc.dram_tensor("bh", [128, 8], F32, kind="ExternalInput")
    bd1 = nc.dram_tensor("bd1", [128, 16], F32, kind="ExternalInput")
    bd2 = nc.dram_tensor("bd2", [128, 8], F32, kind="ExternalInput")
    outD = nc.dram_tensor("out", [OUT, ROWS], F32, kind="ExternalOutput")

    with tile.TileContext(nc) as tc:
        with tc.tile_pool(name="dram", bufs=1, space="DRAM") as dram:
            # dec indexed by pass-B chunk; pass-A chunks write half each
            dec = dram.tile([NCHB, 128, 16, RTB], mm_dtype)

            # outer-scope pools: dec prefetch + the first wd1 columns live in
            # virgin SBUF, so they stream in mid-pass-A with no WAR waits and
            # the pass transition costs ~no tensor idle
            with (
                tc.tile_pool(name="pdc", bufs=2) as pdc,
                tc.tile_pool(name="pwh1", bufs=1) as pwh1,
            ):
                dcs = {}

                def load_dc(chb):
                    t = pdc.tile([128, 16, RTB], mm_dtype, tag="dc", name="dc")
                    nc.sync.dma_start(t, dec[chb])
                    dcs[chb] = t

                WD1H = 4  # wd1 columns preloaded during pass A
                wd1h = pwh1.tile([128, WD1H, 16, 128], mm_dtype, tag="wd1h")

                # ================= PASS A =================
                with (
                    tc.tile_pool(name="pwA", bufs=1) as pwA,
                    tc.tile_pool(name="pxt", bufs=3) as pxt,
                    tc.tile_pool(name="pP", bufs=3) as pP,
                    tc.tile_pool(name="plast", bufs=1) as plast,
                    tc.tile_pool(name="pscr", bufs=3) as pscr,
                    tc.tile_pool(name="psm", bufs=1) as psm,
                    tc.tile_pool(name="pdcx", bufs=1) as pdcx,
                    tc.tile_pool(name="psA", bufs=3, space="PSUM") as psA,
                    tc.tile_pool(name="psC", bufs=3, space="PSUM") as psC,
                ):
                    # -- resident weights / biases --
                    b_sb = []
                    for i in range(3):
                        b = pwA.tile([128, 8], F32, tag=f"b{i}", name=f"b{i}")
                        nc.sync.dma_start(b, bv[i].ap())
                        b_sb.append(b)
                    bh_sb = pwA.tile([128, 8], F32, tag="bh")
                    nc.sync.dma_start(bh_sb, bh.ap())

                    w_sb = []
                    for i in range(3):
                        w = pwA.tile([128, 8, 4, 128], mm_dtype, tag=f"w{i}",
                                     name=f"w{i}")
                        for fc in range(8):
                            nc.scalar.dma_start(w[:, fc], wv[i].ap()[fc])
                        w_sb.append(w)
                    wh_sb = pwA.tile([128, 8, 24, 128], mm_dtype, tag="wh")
                    for fc in range(8):
                        nc.scalar.dma_start(wh_sb[:, fc], wh.ap()[fc])

                    ones_sb = pwA.tile([128, 128], mm_dtype, tag="ones")
                    nc.any.memset(ones_sb, 1.0)

                    xts = {}

                    def load_xt(ch, split=False):
                        t = pxt.tile([128, 12, RTA], mm_dtype, tag="xt",
                                     name="xt")
                        if split:
                            for i in range(3):
                                nc.sync.dma_start(
                                    t[:, 4 * i:4 * i + 4, :],
                                    xT.ap()[ch, :, 4 * i:4 * i + 4, :])
                        else:
                            nc.sync.dma_start(t, xT.ap()[ch])
                        xts[ch] = t

                    load_xt(0, split=True)
                    load_xt(1)
                    load_xt(2)

                    carry = {}

                    def emit_attention(ch):
                        """alphas -> softmax -> ctx -> dec writes, chunk ch."""
                        pt, last, tmps = carry.pop(ch)
                        aps = []
                        for i in range(3):
                            ap_i = psC.tile([128, RTA], F32, tag="alpha",
                                            name=f"alpha{i}")
                            for fc in range(8):
                                nc.tensor.matmul(ap_i, ones_sb,
                                                 tmps[i][:, fc, :],
                                                 start=(fc == 0),
                                                 stop=(fc == 7))
                            aps.append(ap_i)
                        # softmax over 3 logits, all contiguous 2D ops.
                        # (DVE may read at most one PSUM operand, so stage
                        # the logits into SBUF on ScalarE first.)
                        a = []
                        for i in range(3):
                            a_i = psm.tile([128, RTA], F32, tag=f"a{i}",
                                           name=f"a{i}")
                            nc.scalar.copy(a_i, aps[i])
                            a.append(a_i)
                        mx = psm.tile([128, RTA], F32, tag="mx")
                        nc.vector.tensor_tensor(mx, a[0], a[1], AluOp.max)
                        nc.vector.tensor_tensor(mx, mx, a[2], AluOp.max)
                        for i in range(3):
                            nc.vector.tensor_tensor(a[i], a[i], mx,
                                                    AluOp.subtract)
                            nc.scalar.activation(a[i], a[i], Act.Exp)
                        ssum = psm.tile([128, RTA], F32, tag="ssum")
                        nc.vector.tensor_tensor(ssum, a[0], a[1], AluOp.add)
                        nc.vector.tensor_tensor(ssum, ssum, a[2], AluOp.add)
                        rcp = psm.tile([128, RTA], F32, tag="rcp")
                        nc.vector.reciprocal(rcp, ssum)
                        ws = []
                        for i in range(3):
                            ws_i = psm.tile([128, RTA], mm_dtype,
                                            tag=f"ws{i}", name=f"ws{i}")
                            nc.vector.tensor_tensor(ws_i, a[i], rcp,
                                                    AluOp.mult)
                            ws.append(ws_i)
                        # ctx = sum_i ws_i * P_i — per-fc contiguous 2D ops
                        dcx = pdcx.tile([128, 8, RTA], mm_dtype, tag="dcx")
                        t2 = pscr.tile([128, 8, RTA], mm_dtype, tag="scr",
                                       name="t2")
                        for fc in range(8):
                            nc.vector.tensor_tensor(
                                dcx[:, fc, :], ws[0], pt[0][:, fc, :],
                                AluOp.mult)
                            nc.vector.tensor_tensor(
                                t2[:, fc, :], ws[1], pt[1][:, fc, :],
                                AluOp.mult)
                            nc.vector.tensor_tensor(
                                dcx[:, fc, :], dcx[:, fc, :], t2[:, fc, :],
                                AluOp.add)
                            nc.vector.tensor_tensor(
                                t2[:, fc, :], ws[2], pt[2][:, fc, :],
                                AluOp.mult)
                            nc.vector.tensor_tensor(
                                dcx[:, fc, :], dcx[:, fc, :], t2[:, fc, :],
                                AluOp.add)
                        rh = slice((ch % 2) * RTA, (ch % 2) * RTA + RTA)
                        nc.gpsimd.dma_start(dec[ch // 2, :, 0:8, rh], dcx)

                    def emit_l1_group(i, ch):
                        p_i = pP.tile([128, 8, RTA], mm_dtype,
                                      tag=f"p{i}", name=f"p{i}")
                        xt = xts[ch]
                        for fc in range(8):
                            ps = psA.tile([128, RTA], F32, tag="mm",
                                          name="ps1")
                            for kc in range(4):
                                nc.tensor.matmul(
                                    ps, w_sb[i][:, fc, kc, :],
                                    xt[:, 4 * i + kc, :],
                                    start=(kc == 0), stop=(kc == 3))
                            nc.scalar.activation(
                                p_i[:, fc, :], ps, Act.Relu,
                                bias=b_sb[i][:, fc:fc + 1])
                        return p_i

                    def emit_l2_att(ch, pt):
                        last = plast.tile([128, 8, RTA], mm_dtype, tag="last")
                        for fc in range(8):
                            ps = psA.tile([128, RTA], F32, tag="mm",
                                          name="ps2")
                            for i in range(3):
                                for kc in range(8):
                                    nc.tensor.matmul(
                                        ps, wh_sb[:, fc, 8 * i + kc, :],
                                        pt[i][:, kc, :],
                                        start=(i == 0 and kc == 0),
                                        stop=(i == 2 and kc == 7))
                            nc.scalar.activation(last[:, fc, :], ps, Act.Relu,
                                                 bias=bh_sb[:, fc:fc + 1])
                        rh = slice((ch % 2) * RTA, (ch % 2) * RTA + RTA)
                        nc.gpsimd.dma_start(dec[ch // 2, :, 8:16, rh], last)
                        tmps = []
                        for i in range(3):
                            tmp = pscr.tile([128, 8, RTA], mm_dtype,
                                            tag="scr", name=f"tmp{i}")
                            nc.vector.tensor_tensor(tmp, last, pt[i],
                                                    AluOp.mult)
                            tmps.append(tmp)
                        carry[ch] = (pt, last, tmps)

                    # --- warm-up: first 3 chunks run L1 in (i, chunk) order
                    # so compute reuses each weight tensor across 3 chunks
                    # the moment it lands (the weight stream can't keep up
                    # with a single chunk's consumption rate at t=0) ---
                    WARM = 3
                    ptc = {ch: [] for ch in range(WARM)}
                    for i in range(3):
                        for ch in range(WARM):
                            ptc[ch].append(emit_l1_group(i, ch))
                    for ch in range(WARM):
                        xts.pop(ch)
                    load_xt(3)
                    load_xt(4)
                    for ch in range(WARM):
                        emit_l2_att(ch, ptc.pop(ch))
                        if ch == 0:
                            # wd1 head-start columns: virgin SBUF, issued on
                            # the gpsimd ring after dec(0) so the transfer
                            # stays clear of the startup-critical window
                            for fc in range(WD1H):
                                nc.gpsimd.dma_start(wd1h[:, fc],
                                                    wd1.ap()[fc])
                        if ch >= 1:
                            emit_attention(ch - 1)

                    # --- steady state ---
                    for ch in range(WARM, NCHA):
                        pt = [emit_l1_group(i, ch) for i in range(3)]
                        if ch + 2 < NCHA:
                            load_xt(ch + 2)
                        emit_attention(ch - 1)
                        emit_l2_att(ch, pt)

                    emit_attention(NCHA - 1)

                # ================= PASS B =================
                with (
                    tc.tile_pool(name="pwB", bufs=1) as pwB,
                    tc.tile_pool(name="po1", bufs=2) as po1,
                    tc.tile_pool(name="pev", bufs=3) as pev,
                    tc.tile_pool(name="psF", bufs=3, space="PSUM") as psF,
                    tc.tile_pool(name="psG", bufs=3, space="PSUM") as psG,
                ):
                    bd1_sb = pwB.tile([128, 16], F32, tag="bd1")
                    nc.sync.dma_start(bd1_sb, bd1.ap())
                    bd2_sb = pwB.tile([128, 8], F32, tag="bd2")
                    nc.sync.dma_start(bd2_sb, bd2.ap())

                    load_dc(0)
                    load_dc(1)
                    # pass-B weights on the sync queue: it drains early in
                    # pass A, so these stream in as soon as their aliased
                    # SBUF (pass-A tiles) is released
                    wd1_sb = pwB.tile([128, 16 - WD1H, 16, 128], mm_dtype,
                                      tag="wd1")
                    wd2_sb = pwB.tile([128, 8, 16, 128], mm_dtype, tag="wd2")
                    for fc in range(WD1H, 16):
                        nc.sync.dma_start(wd1_sb[:, fc - WD1H], wd1.ap()[fc])
                    for fc in range(8):
                        nc.sync.dma_start(wd2_sb[:, fc], wd2.ap()[fc])

                    for ch in range(NCHB):
                        rs = slice(ch * RTB, (ch + 1) * RTB)
                        dc = dcs.pop(ch)
                        o1 = po1.tile([128, 16, RTB], mm_dtype, tag="o1")
                        for fc in range(16):
                            w1src = (wd1h[:, fc] if fc < WD1H
                                     else wd1_sb[:, fc - WD1H])
                            ps = psF.tile([128, RTB], F32, tag="f1")
                            for kc in range(16):
                                nc.tensor.matmul(ps, w1src[:, kc, :],
                                                 dc[:, kc, :],
                                                 start=(kc == 0),
                                                 stop=(kc == 15))
                            nc.scalar.activation(o1[:, fc, :], ps, Act.Relu,
                                                 bias=bd1_sb[:, fc:fc + 1])
                        # dc(ch) fully consumed; recycle its buffer
                        if ch + 2 < NCHB:
                            load_dc(ch + 2)
                        for oc in range(8):
                            ow = 128 if oc < 7 else OUT - 7 * 128
                            ps = psG.tile([128, RTB], F32, tag="f2")
                            for kc in range(16):
                                nc.tensor.matmul(ps, wd2_sb[:, oc, kc, :],
                                                 o1[:, kc, :],
                                                 start=(kc == 0),
                                                 stop=(kc == 15))
                            ev = pev.tile([128, RTB], F32, tag="ev")
                            nc.vector.tensor_scalar_add(ev, ps,
                                                        bd2_sb[:, oc:oc + 1])
                            nc.gpsimd.dma_start(
                                outD.ap()[oc * 128:oc * 128 + ow, rs],
                                ev[:ow])

    nc.finalize()
    return nc


def _wpack(W, FC, KC, np_dt):
    """[F_out, K_in] -> [FC, 128p, KC, 128f] (pad F_out up to FC*128)."""
    W = np.asarray(W, np.float32)
    F, K = W.shape
    if F < FC * 128:
        W = np.pad(W, ((0, FC * 128 - F), (0, 0)))
    W4 = W.reshape(FC, 128, KC, 128)          # [fc, f, kc, p]
    return np.ascontiguousarray(W4.transpose(0, 3, 2, 1)).astype(np_dt)


def _bmat(b, cc):
    """[F] -> [128, cc] so column c holds b[c*128:(c+1)*128]."""
    b = np.asarray(b, np.float32)
    if b.shape[0] < cc * 128:
        b = np.pad(b, (0, cc * 128 - b.shape[0]))
    return np.ascontiguousarray(b.reshape(cc, 128).T)


def _prep_inputs(np_dt, tube, w1_W, w1_b, w2_W, w2_b, w3_W, w3_b, wh_W, wh_b,
                 wd1_W, wd1_b, wd2_W, wd2_b):
    shared = {
        "w1": _wpack(w1_W, 8, 4, np_dt), "w2": _wpack(w2_W, 8, 4, np_dt),
        "w3": _wpack(w3_W, 8, 4, np_dt), "wh": _wpack(wh_W, 8, 24, np_dt),
        "wd1": _wpack(wd1_W, 16, 16, np_dt), "wd2": _wpack(wd2_W, 8, 16, np_dt),
        "b1": _bmat(w1_b, 8), "b2": _bmat(w2_b, 8), "b3": _bmat(w3_b, 8),
        "bh": _bmat(wh_b, 8), "bd1": _bmat(wd1_b, 16), "bd2": _bmat(wd2_b, 8),
    }
    tube = np.asarray(tube, np.float32)
    in_maps = []
    for c in range(N_CORES):
        t = tube[c * ROWS:(c + 1) * ROWS]                # [ROWS, 1536]
        xc = np.ascontiguousarray(
            t.reshape(NCHA, RTA, 12, 128).transpose(0, 3, 2, 1)
        ).astype(np_dt)                                  # [NCHA, 128, 12, RTA]
        in_maps.append({"xT": xc, **shared})
    return in_maps


_NC_CACHE = {}


def run(inputs, mm_dtype=BF16, trace=False):
    key = (mm_dtype,)
    if key not in _NC_CACHE:
        _NC_CACHE[key] = build_nc(mm_dtype)
    nc = _NC_CACHE[key]
    np_dt = ml_dtypes.bfloat16 if mm_dtype == BF16 else np.float32
    in_maps = _prep_inputs(np_dt, **inputs)
    res = run_bass_kernel_spmd(nc, in_maps, list(range(N_CORES)), trace=trace)
    out = np.empty((B, OUT), np.float32)
    for c in range(N_CORES):
        out[c * ROWS:(c + 1) * ROWS] = res.results[c]["out"].T
    return out, res


def kernel(**inputs) -> np.ndarray:
    out, _ = run(inputs)
    return out


# revision 13
# speedup vs baseline: 1.5417x; 1.0012x over previous
"""Trainium2 Bass kernel for nn_ClassAtt (dense MLP + 3-way class attention).

Model (per row of tube [B, 1536]):
  x1,x2,x3 = tube split into 3x512
  P_i   = relu(x_i @ w_i.T + b_i)            [B, 1024]
  last  = relu(concat(P1,P2,P3) @ wh.T + bh) [B, 1024]
  a_i   = rowwise_dot(last, P_i); w = softmax(a)  [B, 3]
  ctx   = sum_i w_i * P_i                    [B, 1024]
  out   = relu(concat(ctx, last) @ wd1.T + bd1) @ wd2.T + bd2  [B, 1000]

Strategy: pure data parallel over 8 NeuronCores (2048 rows each).  All
activations live in transposed [feature, row] layout so the contraction dim
is always on SBUF partitions and biases are per-partition scalars.  Matmuls
run in bf16 (rel-err budget is 2e-2; bf16 lands ~6e-3) with fp32 PSUM
accumulation, which halves DMA/SBUF traffic vs f32r at the same PE rate
(1 cycle/row).  All weights are SBUF-resident, packed host-side as
[fc, 128p, kc, 128f] so each output-column chunk is one contiguous DMA and
a matmul group can start as soon as its column arrives.

Two fused passes, one DRAM spill between them:
  Pass A (8 chunks of 256 rows): L1, L2, attention + context in SBUF;
          spill dec = [ctx; last] (bf16).
  Pass B (4 chunks of 512 rows): out = relu(dec @ wd1.T + bd1) @ wd2.T
          + bd2 with the full 2048 contraction on-chip.
Attention's partition reduction uses a ones[128,128] matmul (output is
partition-redundant, which doubles as the broadcast for ctx).  All
attention elementwise ops are contiguous 2D [128, R] — broadcast APs and
large GpSimd ops measured 4x slower on DVE/GpSimd and are avoided.
Softmax runs in PSUM (exp on ScalarE).  Emission is software-pipelined:
chunk k's attention matmuls are emitted between L1 and L2 of chunk k+1.
Pass-B weights stream on the sync DMA queue (idle during pass A tail) and
dec row-chunks prefetch into an outer-scope pool so the pass transition
costs ~no tensor idle.
"""

import numpy as np
import ml_dtypes

import concourse.bass as bass
import concourse.mybir as mybir
import concourse.tile as tile
from concourse import bacc
from concourse.bass_utils import run_bass_kernel_spmd

F32 = mybir.dt.float32
F32R = mybir.dt.float32r
BF16 = mybir.dt.bfloat16

N_CORES = 8
B = 16384
ROWS = B // N_CORES   # rows per core
RTA = 256             # pass-A rows per chunk
NCHA = ROWS // RTA
RTB = 512             # pass-B rows per chunk
NCHB = ROWS // RTB
M = 1024              # hidden width
DEC_H = 2048
OUT = 1000

AluOp = mybir.AluOpType
Act = mybir.ActivationFunctionType


def build_nc(mm_dtype=BF16):
    nc = bacc.Bacc(None, target_bir_lowering=False)

    # ---- DRAM I/O (per-core shapes) ----
    # x: [chunk][partition, kchunk, row]  (contiguous per partition)
    xT = nc.dram_tensor("xT", [NCHA, 128, 12, RTA], mm_dtype,
                        kind="ExternalInput")
    # weights: [fc, 128p, kc, 128f] — one contiguous chunk per output column
    wv = [
        nc.dram_tensor(f"w{i + 1}", [8, 128, 4, 128], mm_dtype,
                       kind="ExternalInput")
        for i in range(3)
    ]
    wh = nc.dram_tensor("wh", [8, 128, 24, 128], mm_dtype, kind="ExternalInput")
    wd1 = nc.dram_tensor("wd1", [16, 128, 16, 128], mm_dtype,
                         kind="ExternalInput")
    wd2 = nc.dram_tensor("wd2", [8, 128, 16, 128], mm_dtype,
                         kind="ExternalInput")
    bv = [
        nc.dram_tensor(f"b{i + 1}", [128, 8], F32, kind="ExternalInput")
        for i in range(3)
    ]
    bh = nc.dram_tensor("bh", [128, 8], F32, kind="ExternalInput")
    bd1 = nc.dram_tensor("bd1", [128, 16], F32, kind="ExternalInput")
    bd2 = nc.dram_tensor("bd2", [128, 8], F32, kind="ExternalInput")
    outD = nc.dram_tensor("out", [OUT, ROWS], F32, kind="ExternalOutput")

    with tile.TileContext(nc) as tc:
        with tc.tile_pool(name="dram", bufs=1, space="DRAM") as dram:
            # dec indexed by pass-B chunk; pass-A chunks write half each
            dec = dram.tile([NCHB, 128, 16, RTB], mm_dtype)

            # outer-scope pools: dec prefetch + the first wd1 columns live in
            # virgin SBUF, so they stream in mid-pass-A with no WAR waits and
            # the pass transition costs ~no tensor idle
            with (
                tc.tile_pool(name="pdc", bufs=2) as pdc,
                tc.tile_pool(name="pwh1", bufs=1) as pwh1,
            ):
                dcs = {}

                def load_dc(chb):
                    t = pdc.tile([128, 16, RTB], mm_dtype, tag="dc", name="dc")
                    for q in range(4):
                        cs = slice(4 * q, 4 * q + 4)
                        nc.sync.dma_start(t[:, cs], dec[chb, :, cs])
                    dcs[chb] = t

                WD1H = 4  # wd1 columns preloaded during pass A
                wd1h = pwh1.tile([128, WD1H, 16, 128], mm_dtype, tag="wd1h")

                # ================= PASS A =================
                with (
                    tc.tile_pool(name="pwA", bufs=1) as pwA,
                    tc.tile_pool(name="pxt", bufs=3) as pxt,
                    tc.tile_pool(name="pP", bufs=3) as pP,
                    tc.tile_pool(name="plast", bufs=1) as plast,
                    tc.tile_pool(name="pscr", bufs=3) as pscr,
                    tc.tile_pool(name="psm", bufs=1) as psm,
                    tc.tile_pool(name="pdcx", bufs=1) as pdcx,
                    tc.tile_pool(name="psA", bufs=3, space="PSUM") as psA,
                    tc.tile_pool(name="psC", bufs=3, space="PSUM") as psC,
                ):
                    # -- resident weights / biases --
                    b_sb = []
                    for i in range(3):
                        b = pwA.tile([128, 8], F32, tag=f"b{i}", name=f"b{i}")
                        nc.sync.dma_start(b, bv[i].ap())
                        b_sb.append(b)
                    bh_sb = pwA.tile([128, 8], F32, tag="bh")
                    nc.sync.dma_start(bh_sb, bh.ap())

                    w_sb = []
                    for i in range(3):
                        w = pwA.tile([128, 8, 4, 128], mm_dtype, tag=f"w{i}",
                                     name=f"w{i}")
                        for fc in range(8):
                            nc.scalar.dma_start(w[:, fc], wv[i].ap()[fc])
                        w_sb.append(w)
                    wh_sb = pwA.tile([128, 8, 24, 128], mm_dtype, tag="wh")
                    for fc in range(8):
                        nc.scalar.dma_start(wh_sb[:, fc], wh.ap()[fc])

                    ones_sb = pwA.tile([128, 128], mm_dtype, tag="ones")
                    nc.any.memset(ones_sb, 1.0)

                    xts = {}

                    def load_xt(ch):
                        # one DMA instruction ~= one HW queue (~20GB/s), so
                        # split per x_i to get 3 queues pulling in parallel
                        t = pxt.tile([128, 12, RTA], mm_dtype, tag="xt",
                                     name="xt")
                        for i in range(3):
                            nc.sync.dma_start(
                                t[:, 4 * i:4 * i + 4, :],
                                xT.ap()[ch, :, 4 * i:4 * i + 4, :])
                        xts[ch] = t

                    load_xt(0)
                    load_xt(1)
                    load_xt(2)

                    carry = {}

                    def emit_attention(ch):
                        """alphas -> softmax -> ctx -> dec writes, chunk ch."""
                        pt, last, tmps = carry.pop(ch)
                        aps = []
                        for i in range(3):
                            ap_i = psC.tile([128, RTA], F32, tag="alpha",
                                            name=f"alpha{i}")
                            for fc in range(8):
                                nc.tensor.matmul(ap_i, ones_sb,
                                                 tmps[i][:, fc, :],
                                                 start=(fc == 0),
                                                 stop=(fc == 7))
                            aps.append(ap_i)
                        # softmax over 3 logits, all contiguous 2D ops.
                        # (DVE may read at most one PSUM operand, so stage
                        # the logits into SBUF on ScalarE first.)
                        a = []
                        for i in range(3):
                            a_i = psm.tile([128, RTA], F32, tag=f"a{i}",
                                           name=f"a{i}")
                            nc.scalar.copy(a_i, aps[i])
                            a.append(a_i)
                        mx = psm.tile([128, RTA], F32, tag="mx")
                        nc.vector.tensor_tensor(mx, a[0], a[1], AluOp.max)
                        nc.vector.tensor_tensor(mx, mx, a[2], AluOp.max)
                        for i in range(3):
                            nc.vector.tensor_tensor(a[i], a[i], mx,
                                                    AluOp.subtract)
                            nc.scalar.activation(a[i], a[i], Act.Exp)
                        ssum = psm.tile([128, RTA], F32, tag="ssum")
                        nc.vector.tensor_tensor(ssum, a[0], a[1], AluOp.add)
                        nc.vector.tensor_tensor(ssum, ssum, a[2], AluOp.add)
                        rcp = psm.tile([128, RTA], F32, tag="rcp")
                        nc.vector.reciprocal(rcp, ssum)
                        ws = []
                        for i in range(3):
                            ws_i = psm.tile([128, RTA], mm_dtype,
                                            tag=f"ws{i}", name=f"ws{i}")
                            nc.vector.tensor_tensor(ws_i, a[i], rcp,
                                                    AluOp.mult)
                            ws.append(ws_i)
                        # ctx = sum_i ws_i * P_i — per-fc contiguous 2D ops
                        dcx = pdcx.tile([128, 8, RTA], mm_dtype, tag="dcx")
                        t2 = pscr.tile([128, 8, RTA], mm_dtype, tag="scr",
                                       name="t2")
                        for fc in range(8):
                            nc.vector.tensor_tensor(
                                dcx[:, fc, :], ws[0], pt[0][:, fc, :],
                                AluOp.mult)
                            nc.vector.tensor_tensor(
                                t2[:, fc, :], ws[1], pt[1][:, fc, :],
                                AluOp.mult)
                            nc.vector.tensor_tensor(
                                dcx[:, fc, :], dcx[:, fc, :], t2[:, fc, :],
                                AluOp.add)
                            nc.vector.tensor_tensor(
                                t2[:, fc, :], ws[2], pt[2][:, fc, :],
                                AluOp.mult)
                            nc.vector.tensor_tensor(
                                dcx[:, fc, :], dcx[:, fc, :], t2[:, fc, :],
                                AluOp.add)
                        rh = slice((ch % 2) * RTA, (ch % 2) * RTA + RTA)
                        nc.gpsimd.dma_start(dec[ch // 2, :, 0:8, rh], dcx)

                    def emit_l1_group(i, ch):
                        p_i = pP.tile([128, 8, RTA], mm_dtype,
                                      tag=f"p{i}", name=f"p{i}")
                        xt = xts[ch]
                        for fc in range(8):
                            ps = psA.tile([128, RTA], F32, tag="mm",
                                          name="ps1")
                            for kc in range(4):
                                nc.tensor.matmul(
                                    ps, w_sb[i][:, fc, kc, :],
                                    xt[:, 4 * i + kc, :],
                                    start=(kc == 0), stop=(kc == 3))
                            nc.scalar.activation(
                                p_i[:, fc, :], ps, Act.Relu,
                                bias=b_sb[i][:, fc:fc + 1])
                        return p_i

                    def emit_l2_att(ch, pt):
                        last = plast.tile([128, 8, RTA], mm_dtype, tag="last")
                        for fc in range(8):
                            ps = psA.tile([128, RTA], F32, tag="mm",
                                          name="ps2")
                            for i in range(3):
                                for kc in range(8):
                                    nc.tensor.matmul(
                                        ps, wh_sb[:, fc, 8 * i + kc, :],
                                        pt[i][:, kc, :],
                                        start=(i == 0 and kc == 0),
                                        stop=(i == 2 and kc == 7))
                            nc.scalar.activation(last[:, fc, :], ps, Act.Relu,
                                                 bias=bh_sb[:, fc:fc + 1])
                        rh = slice((ch % 2) * RTA, (ch % 2) * RTA + RTA)
                        nc.gpsimd.dma_start(dec[ch // 2, :, 8:16, rh], last)
                        tmps = []
                        for i in range(3):
                            tmp = pscr.tile([128, 8, RTA], mm_dtype,
                                            tag="scr", name=f"tmp{i}")
                            nc.vector.tensor_tensor(tmp, last, pt[i],
                                                    AluOp.mult)
                            tmps.append(tmp)
                        carry[ch] = (pt, last, tmps)

                    # --- warm-up: first 3 chunks run L1 in (i, chunk) order
                    # so compute reuses each weight tensor across 3 chunks
                    # the moment it lands (the weight stream can't keep up
                    # with a single chunk's consumption rate at t=0) ---
                    WARM = 3
                    ptc = {ch: [] for ch in range(WARM)}
                    for i in range(3):
                        for ch in range(WARM):
                            ptc[ch].append(emit_l1_group(i, ch))
                    for ch in range(WARM):
                        xts.pop(ch)
                    load_xt(3)
                    load_xt(4)
                    for ch in range(WARM):
                        emit_l2_att(ch, ptc.pop(ch))
                        if ch == 0:
                            # wd1 head-start columns: virgin SBUF, issued on
                            # the gpsimd ring after dec(0) so the transfer
                            # stays clear of the startup-critical window
                            for fc in range(WD1H):
                                nc.gpsimd.dma_start(wd1h[:, fc],
                                                    wd1.ap()[fc])
                        if ch >= 1:
                            emit_attention(ch - 1)

                    # --- steady state ---
                    for ch in range(WARM, NCHA):
                        pt = [emit_l1_group(i, ch) for i in range(3)]
                        if ch + 2 < NCHA:
                            load_xt(ch + 2)
                        emit_attention(ch - 1)
                        emit_l2_att(ch, pt)

                    emit_attention(NCHA - 1)

                # ================= PASS B =================
                with (
                    tc.tile_pool(name="pwB", bufs=1) as pwB,
                    tc.tile_pool(name="po1", bufs=2) as po1,
                    tc.tile_pool(name="pev", bufs=3) as pev,
                    tc.tile_pool(name="psF", bufs=3, space="PSUM") as psF,
                    tc.tile_pool(name="psG", bufs=3, space="PSUM") as psG,
                ):
                    bd1_sb = pwB.tile([128, 16], F32, tag="bd1")
                    nc.sync.dma_start(bd1_sb, bd1.ap())
                    bd2_sb = pwB.tile([128, 8], F32, tag="bd2")
                    nc.sync.dma_start(bd2_sb, bd2.ap())

                    load_dc(0)
                    load_dc(1)
                    # pass-B weights on the sync queue: it drains early in
                    # pass A, so these stream in as soon as their aliased
                    # SBUF (pass-A tiles) is released
                    wd1_sb = pwB.tile([128, 16 - WD1H, 16, 128], mm_dtype,
                                      tag="wd1")
                    wd2_sb = pwB.tile([128, 8, 16, 128], mm_dtype, tag="wd2")
                    for fc in range(WD1H, 16):
                        for kh in range(2):
                            ks = slice(8 * kh, 8 * kh + 8)
                            nc.sync.dma_start(wd1_sb[:, fc - WD1H, ks],
                                              wd1.ap()[fc, :, ks])
                    for fc in range(8):
                        nc.sync.dma_start(wd2_sb[:, fc], wd2.ap()[fc])

                    for ch in range(NCHB):
                        rs = slice(ch * RTB, (ch + 1) * RTB)
                        dc = dcs.pop(ch)
                        o1 = po1.tile([128, 16, RTB], mm_dtype, tag="o1")
                        for fc in range(16):
                            w1src = (wd1h[:, fc] if fc < WD1H
                                     else wd1_sb[:, fc - WD1H])
                            ps = psF.tile([128, RTB], F32, tag="f1")
                            for kc in range(16):
                                nc.tensor.matmul(ps, w1src[:, kc, :],
                                                 dc[:, kc, :],
                                                 start=(kc == 0),
                                                 stop=(kc == 15))
                            nc.scalar.activation(o1[:, fc, :], ps, Act.Relu,
                                                 bias=bd1_sb[:, fc:fc + 1])
                        # dc(ch) fully consumed; recycle its buffer
                        if ch + 2 < NCHB:
                            load_dc(ch + 2)
                        for oc in range(8):
                            ow = 128 if oc < 7 else OUT - 7 * 128
                            ps = psG.tile([128, RTB], F32, tag="f2")
                            for kc in range(16):
                                nc.tensor.matmul(ps, wd2_sb[:, oc, kc, :],
                                                 o1[:, kc, :],
                                                 start=(kc == 0),
                                                 stop=(kc == 15))
                            ev = pev.tile([128, RTB], F32, tag="ev")
                            nc.vector.tensor_scalar_add(ev, ps,
                                                        bd2_sb[:, oc:oc + 1])
                            for rh in range(2):
                                rr = slice(ch * RTB + rh * (RTB // 2),
                                           ch * RTB + (rh + 1) * (RTB // 2))
                                cc = slice(rh * (RTB // 2),
                                           (rh + 1) * (RTB // 2))
                                nc.gpsimd.dma_start(
                                    outD.ap()[oc * 128:oc * 128 + ow, rr],
                                    ev[:ow, cc])

    nc.finalize()
    return nc


def _wpack(W, FC, KC, np_dt):
    """[F_out, K_in] -> [FC, 128p, KC, 128f] (pad F_out up to FC*128)."""
    W = np.asarray(W, np.float32)
    F, K = W.shape
    if F < FC * 128:
        W = np.pad(W, ((0, FC * 128 - F), (0, 0)))
    W4 = W.reshape(FC, 128, KC, 128)          # [fc, f, kc, p]
    return np.ascontiguousarray(W4.transpose(0, 3, 2, 1)).astype(np_dt)


def _bmat(b, cc):
    """[F] -> [128, cc] so column c holds b[c*128:(c+1)*128]."""
    b = np.asarray(b, np.float32)
    if b.shape[0] < cc * 128:
        b = np.pad(b, (0, cc * 128 - b.shape[0]))
    return np.ascontiguousarray(b.reshape(cc, 128).T)


def _prep_inputs(np_dt, tube, w1_W, w1_b, w2_W, w2_b, w3_W, w3_b, wh_W, wh_b,
                 wd1_W, wd1_b, wd2_W, wd2_b):
    shared = {
        "w1": _wpack(w1_W, 8, 4, np_dt), "w2": _wpack(w2_W, 8, 4, np_dt),
        "w3": _wpack(w3_W, 8, 4, np_dt), "wh": _wpack(wh_W, 8, 24, np_dt),
        "wd1": _wpack(wd1_W, 16, 16, np_dt), "wd2": _wpack(wd2_W, 8, 16, np_dt),
        "b1": _bmat(w1_b, 8), "b2": _bmat(w2_b, 8), "b3": _bmat(w3_b, 8),
        "bh": _bmat(wh_b, 8), "bd1": _bmat(wd1_b, 16), "bd2": _bmat(wd2_b, 8),
    }
    tube = np.asarray(tube, np.float32)
    in_maps = []
    for c in range(N_CORES):
        t = tube[c * ROWS:(c + 1) * ROWS]                # [ROWS, 1536]
        xc = np.ascontiguousarray(
            t.reshape(NCHA, RTA, 12, 128).transpose(0, 3, 2, 1)
        ).astype(np_dt)                                  # [NCHA, 128, 12, RTA]
        in_maps.append({"xT": xc, **shared})
    return in_maps


_NC_CACHE = {}


def run(inputs, mm_dtype=BF16, trace=False):
    key = (mm_dtype,)
    if key not in _NC_CACHE:
        _NC_CACHE[key] = build_nc(mm_dtype)
    nc = _NC_CACHE[key]
    np_dt = ml_dtypes.bfloat16 if mm_dtype == BF16 else np.float32
    in_maps = _prep_inputs(np_dt, **inputs)
    res = run_bass_kernel_spmd(nc, in_maps, list(range(N_CORES)), trace=trace)
    out = np.empty((B, OUT), np.float32)
    for c in range(N_CORES):
        out[c * ROWS:(c + 1) * ROWS] = res.results[c]["out"].T
    return out, res


def kernel(**inputs) -> np.ndarray:
    out, _ = run(inputs)
    return out


# revision 17
# speedup vs baseline: 1.5768x; 1.0227x over previous
"""Trainium2 Bass kernel for nn_ClassAtt (dense MLP + 3-way class attention).

Model (per row of tube [B, 1536]):
  x1,x2,x3 = tube split into 3x512
  P_i   = relu(x_i @ w_i.T + b_i)            [B, 1024]
  last  = relu(concat(P1,P2,P3) @ wh.T + bh) [B, 1024]
  a_i   = rowwise_dot(last, P_i); w = softmax(a)  [B, 3]
  ctx   = sum_i w_i * P_i                    [B, 1024]
  out   = relu(concat(ctx, last) @ wd1.T + bd1) @ wd2.T + bd2  [B, 1000]

Strategy: pure data parallel over 8 NeuronCores (2048 rows each).  All
activations live in transposed [feature, row] layout so the contraction dim
is always on SBUF partitions and biases are per-partition scalars.  Matmuls
run in bf16 (rel-err budget is 2e-2; bf16 lands ~6e-3) with fp32 PSUM
accumulation, which halves DMA/SBUF traffic vs f32r at the same PE rate
(1 cycle/row).  All weights are SBUF-resident, packed host-side as
[fc, 128p, kc, 128f] so each output-column chunk is one contiguous DMA and
a matmul group can start as soon as its column arrives.

Two fused passes, one DRAM spill between them:
  Pass A (8 chunks of 256 rows): L1, L2, attention + context in SBUF;
          spill dec = [ctx; last] (bf16).
  Pass B (4 chunks of 512 rows): out = relu(dec @ wd1.T + bd1) @ wd2.T
          + bd2 with the full 2048 contraction on-chip.
Attention's partition reduction uses a ones[128,128] matmul (output is
partition-redundant, which doubles as the broadcast for ctx).  All
attention elementwise ops are contiguous 2D [128, R] — broadcast APs and
large GpSimd ops measured 4x slower on DVE/GpSimd and are avoided.
Softmax runs in PSUM (exp on ScalarE).  Emission is software-pipelined:
chunk k's attention matmuls are emitted between L1 and L2 of chunk k+1.
Pass-B weights stream on the sync DMA queue (idle during pass A tail) and
dec row-chunks prefetch into an outer-scope pool so the pass transition
costs ~no tensor idle.
"""

import numpy as np
import ml_dtypes

import concourse.bass as bass
import concourse.mybir as mybir
import concourse.tile as tile
from concourse import bacc
from concourse.bass_utils import run_bass_kernel_spmd

F32 = mybir.dt.float32
F32R = mybir.dt.float32r
BF16 = mybir.dt.bfloat16

N_CORES = 8
B = 16384
ROWS = B // N_CORES   # rows per core
RTA = 256             # pass-A rows per chunk
NCHA = ROWS // RTA
RTB = 512             # pass-B rows per chunk
NCHB = ROWS // RTB
M = 1024              # hidden width
DEC_H = 2048
OUT = 1000

AluOp = mybir.AluOpType
Act = mybir.ActivationFunctionType


def build_nc(mm_dtype=BF16):
    nc = bacc.Bacc(None, target_bir_lowering=False)

    # ---- DRAM I/O (per-core shapes) ----
    # x: [chunk][partition, kchunk, row]  (contiguous per partition)
    xT = nc.dram_tensor("xT", [NCHA, 128, 12, RTA], mm_dtype,
                        kind="ExternalInput")
    # weights: [fc, 128p, kc, 128f] — one contiguous chunk per output column
    wv = [
        nc.dram_tensor(f"w{i + 1}", [8, 128, 4, 128], mm_dtype,
                       kind="ExternalInput")
        for i in range(3)
    ]
    wh = nc.dram_tensor("wh", [8, 128, 24, 128], mm_dtype, kind="ExternalInput")
    wd1 = nc.dram_tensor("wd1", [16, 128, 16, 128], mm_dtype,
                         kind="ExternalInput")
    wd2 = nc.dram_tensor("wd2", [8, 128, 16, 128], mm_dtype,
                         kind="ExternalInput")
    bv = [
        nc.dram_tensor(f"b{i + 1}", [128, 8], F32, kind="ExternalInput")
        for i in range(3)
    ]
    bh = nc.dram_tensor("bh", [128, 8], F32, kind="ExternalInput")
    bd1 = nc.dram_tensor("bd1", [128, 16], F32, kind="ExternalInput")
    bd2 = nc.dram_tensor("bd2", [128, 8], F32, kind="ExternalInput")
    outD = nc.dram_tensor("out", [OUT, ROWS], F32, kind="ExternalOutput")

    with tile.TileContext(nc) as tc:
        with tc.tile_pool(name="dram", bufs=1, space="DRAM") as dram:
            # dec indexed by pass-B chunk; pass-A chunks write half each
            dec = dram.tile([NCHB, 128, 16, RTB], mm_dtype)

            # outer-scope pools: dec prefetch + the first wd1 columns live in
            # virgin SBUF, so they stream in mid-pass-A with no WAR waits and
            # the pass transition costs ~no tensor idle
            with (
                tc.tile_pool(name="pdc", bufs=2) as pdc,
                tc.tile_pool(name="pwh1", bufs=1) as pwh1,
            ):
                dcs = {}

                def load_dc(chb):
                    t = pdc.tile([128, 16, RTB], mm_dtype, tag="dc", name="dc")
                    for q in range(4):
                        cs = slice(4 * q, 4 * q + 4)
                        nc.sync.dma_start(t[:, cs], dec[chb, :, cs])
                    dcs[chb] = t

                WD1H = 4  # wd1 columns preloaded during pass A
                wd1h = pwh1.tile([128, WD1H, 16, 128], mm_dtype, tag="wd1h")

                # ================= PASS A =================
                with (
                    tc.tile_pool(name="pwA", bufs=1) as pwA,
                    tc.tile_pool(name="pxt", bufs=3) as pxt,
                    tc.tile_pool(name="pP", bufs=3) as pP,
                    tc.tile_pool(name="plast", bufs=1) as plast,
                    tc.tile_pool(name="pscr", bufs=3) as pscr,
                    tc.tile_pool(name="psm", bufs=1) as psm,
                    tc.tile_pool(name="pdcx", bufs=1) as pdcx,
                    tc.tile_pool(name="psC", bufs=3, space="PSUM") as psC,
                ):
                    # psA closes before the final emit_attention so pass B's
                    # PSUM banks release at L2(7) instead of after softmax(7)
                    psA_cm = tc.tile_pool(name="psA", bufs=3, space="PSUM")
                    psA = psA_cm.__enter__()
                    # -- resident weights / biases --
                    # biases + x on the gpsimd ring, weights on the sync ring
                    # — ScalarE's stream must stay pure compute (dma_start
                    # issues backpressure the issuing engine's ring and would
                    # stall the first RELUs ~16us)
                    b_sb = []
                    for i in range(3):
                        b = pwA.tile([128, 8], F32, tag=f"b{i}", name=f"b{i}")
                        nc.gpsimd.dma_start(b, bv[i].ap())
                        b_sb.append(b)
                    bh_sb = pwA.tile([128, 8], F32, tag="bh")
                    nc.gpsimd.dma_start(bh_sb, bh.ap())

                    w_sb = []
                    for i in range(3):
                        w = pwA.tile([128, 8, 4, 128], mm_dtype, tag=f"w{i}",
                                     name=f"w{i}")
                        for fc in range(8):
                            nc.sync.dma_start(w[:, fc], wv[i].ap()[fc])
                        w_sb.append(w)
                    wh_sb = pwA.tile([128, 8, 24, 128], mm_dtype, tag="wh")
                    for fc in range(8):
                        nc.sync.dma_start(wh_sb[:, fc], wh.ap()[fc])

                    ones_sb = pwA.tile([128, 128], mm_dtype, tag="ones")
                    nc.any.memset(ones_sb, 1.0)

                    xts = {}

                    def load_xt(ch, nsplit=3):
                        # one DMA instruction ~= one HW queue (~20GB/s), so
                        # split to get parallel queues pulling
                        t = pxt.tile([128, 12, RTA], mm_dtype, tag="xt",
                                     name="xt")
                        w = 12 // nsplit
                        for i in range(nsplit):
                            nc.gpsimd.dma_start(
                                t[:, w * i:w * i + w, :],
                                xT.ap()[ch, :, w * i:w * i + w, :])
                        xts[ch] = t

                    load_xt(0, nsplit=12)
                    load_xt(1, nsplit=6)
                    load_xt(2, nsplit=6)

                    carry = {}

                    def emit_attention(ch):
                        """alphas -> softmax -> ctx -> dec writes, chunk ch."""
                        pt, last, tmps = carry.pop(ch)
                        aps = []
                        for i in range(3):
                            ap_i = psC.tile([128, RTA], F32, tag="alpha",
                                            name=f"alpha{i}")
                            for fc in range(8):
                                nc.tensor.matmul(ap_i, ones_sb,
                                                 tmps[i][:, fc, :],
                                                 start=(fc == 0),
                                                 stop=(fc == 7))
                            aps.append(ap_i)
                        # softmax over 3 logits, all contiguous 2D ops.
                        # (DVE may read at most one PSUM operand, so stage
                        # the logits into SBUF on ScalarE first.)
                        a = []
                        for i in range(3):
                            a_i = psm.tile([128, RTA], F32, tag=f"a{i}",
                                           name=f"a{i}")
                            nc.scalar.copy(a_i, aps[i])
                            a.append(a_i)
                        mx = psm.tile([128, RTA], F32, tag="mx")
                        nc.vector.tensor_tensor(mx, a[0], a[1], AluOp.max)
                        nc.vector.tensor_tensor(mx, mx, a[2], AluOp.max)
                        for i in range(3):
                            nc.vector.tensor_tensor(a[i], a[i], mx,
                                                    AluOp.subtract)
                            nc.scalar.activation(a[i], a[i], Act.Exp)
                        ssum = psm.tile([128, RTA], F32, tag="ssum")
                        nc.vector.tensor_tensor(ssum, a[0], a[1], AluOp.add)
                        nc.vector.tensor_tensor(ssum, ssum, a[2], AluOp.add)
                        rcp = psm.tile([128, RTA], F32, tag="rcp")
                        nc.vector.reciprocal(rcp, ssum)
                        ws = []
                        for i in range(3):
                            ws_i = psm.tile([128, RTA], mm_dtype,
                                            tag=f"ws{i}", name=f"ws{i}")
                            nc.vector.tensor_tensor(ws_i, a[i], rcp,
                                                    AluOp.mult)
                            ws.append(ws_i)
                        # ctx = sum_i ws_i * P_i — per-fc contiguous 2D ops
                        dcx = pdcx.tile([128, 8, RTA], mm_dtype, tag="dcx")
                        t2 = pscr.tile([128, 8, RTA], mm_dtype, tag="scr",
                                       name="t2")
                        for fc in range(8):
                            nc.vector.tensor_tensor(
                                dcx[:, fc, :], ws[0], pt[0][:, fc, :],
                                AluOp.mult)
                            nc.vector.tensor_tensor(
                                t2[:, fc, :], ws[1], pt[1][:, fc, :],
                                AluOp.mult)
                            nc.vector.tensor_tensor(
                                dcx[:, fc, :], dcx[:, fc, :], t2[:, fc, :],
                                AluOp.add)
                            nc.vector.tensor_tensor(
                                t2[:, fc, :], ws[2], pt[2][:, fc, :],
                                AluOp.mult)
                            nc.vector.tensor_tensor(
                                dcx[:, fc, :], dcx[:, fc, :], t2[:, fc, :],
                                AluOp.add)
                        rh = slice((ch % 2) * RTA, (ch % 2) * RTA + RTA)
                        nc.gpsimd.dma_start(dec[ch // 2, :, 0:8, rh], dcx)

                    def emit_l1_group(i, ch):
                        p_i = pP.tile([128, 8, RTA], mm_dtype,
                                      tag=f"p{i}", name=f"p{i}")
                        xt = xts[ch]
                        for fc in range(8):
                            ps = psA.tile([128, RTA], F32, tag="mm",
                                          name="ps1")
                            for kc in range(4):
                                nc.tensor.matmul(
                                    ps, w_sb[i][:, fc, kc, :],
                                    xt[:, 4 * i + kc, :],
                                    start=(kc == 0), stop=(kc == 3))
                            nc.scalar.activation(
                                p_i[:, fc, :], ps, Act.Relu,
                                bias=b_sb[i][:, fc:fc + 1])
                        return p_i

                    def emit_l2_att(ch, pt):
                        last = plast.tile([128, 8, RTA], mm_dtype, tag="last")
                        for fc in range(8):
                            ps = psA.tile([128, RTA], F32, tag="mm",
                                          name="ps2")
                            for i in range(3):
                                for kc in range(8):
                                    nc.tensor.matmul(
                                        ps, wh_sb[:, fc, 8 * i + kc, :],
                                        pt[i][:, kc, :],
                                        start=(i == 0 and kc == 0),
                                        stop=(i == 2 and kc == 7))
                            nc.scalar.activation(last[:, fc, :], ps, Act.Relu,
                                                 bias=bh_sb[:, fc:fc + 1])
                        rh = slice((ch % 2) * RTA, (ch % 2) * RTA + RTA)
                        nc.gpsimd.dma_start(dec[ch // 2, :, 8:16, rh], last)
                        tmps = []
                        for i in range(3):
                            tmp = pscr.tile([128, 8, RTA], mm_dtype,
                                            tag="scr", name=f"tmp{i}")
                            nc.vector.tensor_tensor(tmp, last, pt[i],
                                                    AluOp.mult)
                            tmps.append(tmp)
                        carry[ch] = (pt, last, tmps)

                    # --- warm-up: first 3 chunks run L1 in (i, chunk) order
                    # so compute reuses each weight tensor across 3 chunks
                    # the moment it lands (the weight stream can't keep up
                    # with a single chunk's consumption rate at t=0) ---
                    WARM = 3
                    ptc = {ch: [] for ch in range(WARM)}
                    for i in range(3):
                        for ch in range(WARM):
                            ptc[ch].append(emit_l1_group(i, ch))
                    for ch in range(WARM):
                        xts.pop(ch)
                    load_xt(3)
                    load_xt(4)
                    for ch in range(WARM):
                        emit_l2_att(ch, ptc.pop(ch))
                        if ch == 0:
                            # wd1 head-start columns: virgin SBUF, issued on
                            # the gpsimd ring after dec(0) so the transfer
                            # stays clear of the startup-critical window
                            for fc in range(WD1H):
                                nc.gpsimd.dma_start(wd1h[:, fc],
                                                    wd1.ap()[fc])
                        if ch >= 1:
                            emit_attention(ch - 1)

                    # --- steady state ---
                    for ch in range(WARM, NCHA):
                        pt = [emit_l1_group(i, ch) for i in range(3)]
                        if ch + 2 < NCHA:
                            load_xt(ch + 2)
                        emit_attention(ch - 1)
                        emit_l2_att(ch, pt)

                    psA_cm.__exit__(None, None, None)
                    emit_attention(NCHA - 1)

                # ================= PASS B =================
                with (
                    tc.tile_pool(name="pwB", bufs=1) as pwB,
                    tc.tile_pool(name="po1", bufs=2) as po1,
                    tc.tile_pool(name="pev", bufs=3) as pev,
                    tc.tile_pool(name="psF", bufs=3, space="PSUM") as psF,
                    tc.tile_pool(name="psG", bufs=3, space="PSUM") as psG,
                ):
                    bd1_sb = pwB.tile([128, 16], F32, tag="bd1")
                    nc.sync.dma_start(bd1_sb, bd1.ap())
                    bd2_sb = pwB.tile([128, 8], F32, tag="bd2")
                    nc.sync.dma_start(bd2_sb, bd2.ap())

                    load_dc(0)
                    load_dc(1)
                    # pass-B weights on the sync queue: it drains early in
                    # pass A, so these stream in as soon as their aliased
                    # SBUF (pass-A tiles) is released
                    wd1_sb = pwB.tile([128, 16 - WD1H, 16, 128], mm_dtype,
                                      tag="wd1")
                    wd2_sb = pwB.tile([128, 8, 16, 128], mm_dtype, tag="wd2")
                    for fc in range(WD1H, 16):
                        for kh in range(2):
                            ks = slice(8 * kh, 8 * kh + 8)
                            nc.sync.dma_start(wd1_sb[:, fc - WD1H, ks],
                                              wd1.ap()[fc, :, ks])
                    for fc in range(8):
                        nc.sync.dma_start(wd2_sb[:, fc], wd2.ap()[fc])

                    for ch in range(NCHB):
                        rs = slice(ch * RTB, (ch + 1) * RTB)
                        dc = dcs.pop(ch)
                        o1 = po1.tile([128, 16, RTB], mm_dtype, tag="o1")
                        for fc in range(16):
                            w1src = (wd1h[:, fc] if fc < WD1H
                                     else wd1_sb[:, fc - WD1H])
                            ps = psF.tile([128, RTB], F32, tag="f1")
                            for kc in range(16):
                                nc.tensor.matmul(ps, w1src[:, kc, :],
                                                 dc[:, kc, :],
                                                 start=(kc == 0),
                                                 stop=(kc == 15))
                            nc.scalar.activation(o1[:, fc, :], ps, Act.Relu,
                                                 bias=bd1_sb[:, fc:fc + 1])
                        # dc(ch) fully consumed; recycle its buffer
                        if ch + 2 < NCHB:
                            load_dc(ch + 2)
                        for oc in range(8):
                            ow = 128 if oc < 7 else OUT - 7 * 128
                            ps = psG.tile([128, RTB], F32, tag="f2")
                            for kc in range(16):
                                nc.tensor.matmul(ps, wd2_sb[:, oc, kc, :],
                                                 o1[:, kc, :],
                                                 start=(kc == 0),
                                                 stop=(kc == 15))
                            ev = pev.tile([128, RTB], F32, tag="ev")
                            nc.vector.tensor_scalar_add(ev, ps,
                                                        bd2_sb[:, oc:oc + 1])
                            for rh in range(2):
                                rr = slice(ch * RTB + rh * (RTB // 2),
                                           ch * RTB + (rh + 1) * (RTB // 2))
                                cc = slice(rh * (RTB // 2),
                                           (rh + 1) * (RTB // 2))
                                nc.gpsimd.dma_start(
                                    outD.ap()[oc * 128:oc * 128 + ow, rr],
                                    ev[:ow, cc])

    nc.finalize()
    return nc


def _wpack(W, FC, KC, np_dt):
    """[F_out, K_in] -> [FC, 128p, KC, 128f] (pad F_out up to FC*128)."""
    W = np.asarray(W, np.float32)
    F, K = W.shape
    if F < FC * 128:
        W = np.pad(W, ((0, FC * 128 - F), (0, 0)))
    W4 = W.reshape(FC, 128, KC, 128)          # [fc, f, kc, p]
    return np.ascontiguousarray(W4.transpose(0, 3, 2, 1)).astype(np_dt)


def _bmat(b, cc):
    """[F] -> [128, cc] so column c holds b[c*128:(c+1)*128]."""
    b = np.asarray(b, np.float32)
    if b.shape[0] < cc * 128:
        b = np.pad(b, (0, cc * 128 - b.shape[0]))
    return np.ascontiguousarray(b.reshape(cc, 128).T)


def _prep_inputs(np_dt, tube, w1_W, w1_b, w2_W, w2_b, w3_W, w3_b, wh_W, wh_b,
                 wd1_W, wd1_b, wd2_W, wd2_b):
    shared = {
        "w1": _wpack(w1_W, 8, 4, np_dt), "w2": _wpack(w2_W, 8, 4, np_dt),
        "w3": _wpack(w3_W, 8, 4, np_dt), "wh": _wpack(wh_W, 8, 24, np_dt),
        "wd1": _wpack(wd1_W, 16, 16, np_dt), "wd2": _wpack(wd2_W, 8, 16, np_dt),
        "b1": _bmat(w1_b, 8), "b2": _bmat(w2_b, 8), "b3": _bmat(w3_b, 8),
        "bh": _bmat(wh_b, 8), "bd1": _bmat(wd1_b, 16), "bd2": _bmat(wd2_b, 8),
    }
    tube = np.asarray(tube, np.float32)
    in_maps = []
    for c in range(N_CORES):
        t = tube[c * ROWS:(c + 1) * ROWS]                # [ROWS, 1536]
        xc = np.ascontiguousarray(
            t.reshape(NCHA, RTA, 12, 128).transpose(0, 3, 2, 1)
        ).astype(np_dt)                                  # [NCHA, 128, 12, RTA]
        in_maps.append({"xT": xc, **shared})
    return in_maps


_NC_CACHE = {}


def run(inputs, mm_dtype=BF16, trace=False):
    key = (mm_dtype,)
    if key not in _NC_CACHE:
        _NC_CACHE[key] = build_nc(mm_dtype)
    nc = _NC_CACHE[key]
    np_dt = ml_dtypes.bfloat16 if mm_dtype == BF16 else np.float32
    in_maps = _prep_inputs(np_dt, **inputs)
    res = run_bass_kernel_spmd(nc, in_maps, list(range(N_CORES)), trace=trace)
    out = np.empty((B, OUT), np.float32)
    for c in range(N_CORES):
        out[c * ROWS:(c + 1) * ROWS] = res.results[c]["out"].T
    return out, res


def kernel(**inputs) -> np.ndarray:
    out, _ = run(inputs)
    return out


# revision 28
# speedup vs baseline: 1.5861x; 1.0059x over previous
"""Trainium2 Bass kernel for nn_ClassAtt (dense MLP + 3-way class attention).

Model (per row of tube [B, 1536]):
  x1,x2,x3 = tube split into 3x512
  P_i   = relu(x_i @ w_i.T + b_i)            [B, 1024]
  last  = relu(concat(P1,P2,P3) @ wh.T + bh) [B, 1024]
  a_i   = rowwise_dot(last, P_i); w = softmax(a)  [B, 3]
  ctx   = sum_i w_i * P_i                    [B, 1024]
  out   = relu(concat(ctx, last) @ wd1.T + bd1) @ wd2.T + bd2  [B, 1000]

Strategy: pure data parallel over 8 NeuronCores (2048 rows each).  All
activations live in transposed [feature, row] layout so the contraction dim
is always on SBUF partitions and biases are per-partition scalars.  Matmuls
run in bf16 (rel-err budget is 2e-2; bf16 lands ~6e-3) with fp32 PSUM
accumulation, which halves DMA/SBUF traffic vs f32r at the same PE rate
(1 cycle/row).  All weights are SBUF-resident, packed host-side as
[fc, 128p, kc, 128f] so each output-column chunk is one contiguous DMA and
a matmul group can start as soon as its column arrives.

Two fused passes, one DRAM spill between them:
  Pass A (8 chunks of 256 rows): L1, L2, attention + context in SBUF;
          spill dec = [ctx; last] (bf16).
  Pass B (4 chunks of 512 rows): out = relu(dec @ wd1.T + bd1) @ wd2.T
          + bd2 with the full 2048 contraction on-chip.
Attention's partition reduction uses a ones[128,128] matmul (output is
partition-redundant, which doubles as the broadcast for ctx).  All
attention elementwise ops are contiguous 2D [128, R] — broadcast APs and
large GpSimd ops measured 4x slower on DVE/GpSimd and are avoided.
Softmax runs in PSUM (exp on ScalarE).  Emission is software-pipelined:
chunk k's attention matmuls are emitted between L1 and L2 of chunk k+1.
Pass-B weights stream on the sync DMA queue (idle during pass A tail) and
dec row-chunks prefetch into an outer-scope pool so the pass transition
costs ~no tensor idle.
"""

import numpy as np
import ml_dtypes

import concourse.bass as bass
import concourse.mybir as mybir
import concourse.tile as tile
from concourse import bacc
from concourse.bass_utils import run_bass_kernel_spmd

F32 = mybir.dt.float32
F32R = mybir.dt.float32r
BF16 = mybir.dt.bfloat16

N_CORES = 8
B = 16384
ROWS = B // N_CORES   # rows per core
RTA = 256             # pass-A rows per chunk
NCHA = ROWS // RTA
RTB = 512             # pass-B rows per chunk
NCHB = ROWS // RTB
M = 1024              # hidden width
DEC_H = 2048
OUT = 1000

AluOp = mybir.AluOpType
Act = mybir.ActivationFunctionType


def build_nc(mm_dtype=BF16):
    nc = bacc.Bacc(None, target_bir_lowering=False)

    # ---- DRAM I/O (per-core shapes) ----
    # x: [chunk][partition, kchunk, row]  (contiguous per partition)
    xT = nc.dram_tensor("xT", [NCHA, 128, 12, RTA], mm_dtype,
                        kind="ExternalInput")
    # weights: [fc, 128p, kc, 128f] — one contiguous chunk per output column
    wv = [
        nc.dram_tensor(f"w{i + 1}", [8, 128, 4, 128], mm_dtype,
                       kind="ExternalInput")
        for i in range(3)
    ]
    wh = nc.dram_tensor("wh", [8, 128, 24, 128], mm_dtype, kind="ExternalInput")
    wd1 = nc.dram_tensor("wd1", [16, 128, 16, 128], mm_dtype,
                         kind="ExternalInput")
    wd2 = nc.dram_tensor("wd2", [8, 128, 16, 128], mm_dtype,
                         kind="ExternalInput")
    bv = [
        nc.dram_tensor(f"b{i + 1}", [128, 8], F32, kind="ExternalInput")
        for i in range(3)
    ]
    bh = nc.dram_tensor("bh", [128, 8], F32, kind="ExternalInput")
    bd1 = nc.dram_tensor("bd1", [128, 16], F32, kind="ExternalInput")
    bd2 = nc.dram_tensor("bd2", [128, 8], F32, kind="ExternalInput")
    outD = nc.dram_tensor("out", [OUT, ROWS], F32, kind="ExternalOutput")

    with tile.TileContext(nc) as tc:
        with tc.tile_pool(name="dram", bufs=1, space="DRAM") as dram:
            # dec indexed by pass-B chunk; pass-A chunks write half each
            dec = dram.tile([NCHB, 128, 16, RTB], mm_dtype)

            # outer-scope pools: dec prefetch + the first wd1 columns live in
            # virgin SBUF, so they stream in mid-pass-A with no WAR waits and
            # the pass transition costs ~no tensor idle
            with (
                tc.tile_pool(name="pdc", bufs=2) as pdc,
                tc.tile_pool(name="pwh1", bufs=1) as pwh1,
            ):
                dcs = {}

                def load_dc(chb):
                    t = pdc.tile([128, 16, RTB], mm_dtype, tag="dc", name="dc")
                    for q in range(4):
                        cs = slice(4 * q, 4 * q + 4)
                        nc.sync.dma_start(t[:, cs], dec[chb, :, cs])
                    dcs[chb] = t

                WD1H = 4  # wd1 columns preloaded during pass A
                wd1h = pwh1.tile([128, WD1H, 16, 128], mm_dtype, tag="wd1h")

                # ================= PASS A =================
                with (
                    tc.tile_pool(name="pwA", bufs=1) as pwA,
                    tc.tile_pool(name="pxt", bufs=3) as pxt,
                    tc.tile_pool(name="pP", bufs=3) as pP,
                    tc.tile_pool(name="plast", bufs=1) as plast,
                    tc.tile_pool(name="pscr", bufs=3) as pscr,
                    tc.tile_pool(name="psm", bufs=1) as psm,
                    tc.tile_pool(name="pdcx", bufs=1) as pdcx,
                    tc.tile_pool(name="psC", bufs=3, space="PSUM",
                                 side="right") as psC,
                ):
                    # psA closes before the final emit_attention so pass B's
                    # PSUM banks release at L2(7) instead of after softmax(7)
                    psA_cm = tc.tile_pool(name="psA", bufs=3, space="PSUM")
                    psA = psA_cm.__enter__()
                    # -- resident weights / biases --
                    # biases + x on the gpsimd ring, weights on the sync ring
                    # — ScalarE's stream must stay pure compute (dma_start
                    # issues backpressure the issuing engine's ring and would
                    # stall the first RELUs ~16us)
                    b_sb = []
                    for i in range(3):
                        b = pwA.tile([128, 8], F32, tag=f"b{i}", name=f"b{i}")
                        nc.gpsimd.dma_start(b, bv[i].ap())
                        b_sb.append(b)
                    bh_sb = pwA.tile([128, 8], F32, tag="bh")
                    nc.gpsimd.dma_start(bh_sb, bh.ap())

                    w_sb = []
                    for i in range(3):
                        w = pwA.tile([128, 8, 4, 128], mm_dtype, tag=f"w{i}",
                                     name=f"w{i}")
                        for fc in range(8):
                            for kh in range(2):
                                ks = slice(2 * kh, 2 * kh + 2)
                                nc.sync.dma_start(w[:, fc, ks],
                                                  wv[i].ap()[fc, :, ks])
                        w_sb.append(w)
                    wh_sb = pwA.tile([128, 8, 24, 128], mm_dtype, tag="wh")
                    for fc in range(8):
                        for kh in range(2):
                            ks = slice(12 * kh, 12 * kh + 12)
                            nc.sync.dma_start(wh_sb[:, fc, ks],
                                              wh.ap()[fc, :, ks])

                    ones_r = pwA.tile([128, 128], F32R, tag="ones")
                    ones_f = psm.tile([128, 128], F32, tag="onesf")
                    nc.any.memset(ones_f, 1.0)
                    nc.vector.tensor_copy(ones_r, ones_f)

                    xts = {}

                    def load_xt(ch, nsplit=3):
                        # one DMA instruction ~= one HW queue (~20GB/s), so
                        # split to get parallel queues pulling
                        t = pxt.tile([128, 12, RTA], mm_dtype, tag="xt",
                                     name="xt")
                        w = 12 // nsplit
                        for i in range(nsplit):
                            nc.gpsimd.dma_start(
                                t[:, w * i:w * i + w, :],
                                xT.ap()[ch, :, w * i:w * i + w, :])
                        xts[ch] = t

                    load_xt(0, nsplit=12)
                    load_xt(1, nsplit=6)
                    load_xt(2, nsplit=6)

                    carry = {}

                    def emit_attention(ch):
                        """alphas -> softmax -> ctx -> dec writes, chunk ch."""
                        pt, last, tmps = carry.pop(ch)
                        # fc-tree in fp32 on DVE (3 adds per i), then a
                        # single ones-matmul per i for the partition sum —
                        # 8x fewer alpha matmuls, DVE has the slack
                        aps = []
                        for i in range(3):
                            s4 = psm.tile([128, 4, RTA], mm_dtype, tag="s4")
                            nc.vector.tensor_tensor(
                                s4, tmps[i][:, 0:4, :], tmps[i][:, 4:8, :],
                                AluOp.add)
                            s2 = psm.tile([128, 2, RTA], mm_dtype, tag="s2")
                            nc.vector.tensor_tensor(
                                s2, s4[:, 0:2, :], s4[:, 2:4, :], AluOp.add)
                            s1 = psm.tile([128, RTA], F32R, tag="s1",
                                          name="s1", bufs=2)
                            nc.vector.tensor_tensor(
                                s1, s2[:, 0, :], s2[:, 1, :], AluOp.add)
                            ap_i = psC.tile([128, RTA], F32, tag="alpha",
                                            name=f"alpha{i}")
                            nc.tensor.matmul(ap_i, ones_r, s1,
                                             start=True, stop=True)
                            aps.append(ap_i)
                        # softmax over 3 logits, all contiguous 2D ops.
                        # (DVE may read at most one PSUM operand, so stage
                        # the logits into SBUF on ScalarE first.)
                        a = []
                        for i in range(3):
                            a_i = psm.tile([128, RTA], F32, tag=f"a{i}",
                                           name=f"a{i}")
                            nc.scalar.copy(a_i, aps[i])
                            a.append(a_i)
                        mx = psm.tile([128, RTA], F32, tag="mx")
                        nc.vector.tensor_tensor(mx, a[0], a[1], AluOp.max)
                        nc.vector.tensor_tensor(mx, mx, a[2], AluOp.max)
                        for i in range(3):
                            nc.vector.tensor_tensor(a[i], a[i], mx,
                                                    AluOp.subtract)
                            nc.scalar.activation(a[i], a[i], Act.Exp)
                        ssum = psm.tile([128, RTA], F32, tag="ssum")
                        nc.vector.tensor_tensor(ssum, a[0], a[1], AluOp.add)
                        nc.vector.tensor_tensor(ssum, ssum, a[2], AluOp.add)
                        rcp = psm.tile([128, RTA], F32, tag="rcp")
                        nc.vector.reciprocal(rcp, ssum)
                        ws = []
                        for i in range(3):
                            ws_i = psm.tile([128, RTA], mm_dtype,
                                            tag=f"ws{i}", name=f"ws{i}")
                            nc.vector.tensor_tensor(ws_i, a[i], rcp,
                                                    AluOp.mult)
                            ws.append(ws_i)
                        # ctx = sum_i ws_i * P_i — per-fc contiguous 2D ops
                        dcx = pdcx.tile([128, 8, RTA], mm_dtype, tag="dcx")
                        t2 = pscr.tile([128, 8, RTA], mm_dtype, tag="scr",
                                       name="t2")
                        for fc in range(8):
                            nc.vector.tensor_tensor(
                                dcx[:, fc, :], ws[0], pt[0][:, fc, :],
                                AluOp.mult)
                            nc.vector.tensor_tensor(
                                t2[:, fc, :], ws[1], pt[1][:, fc, :],
                                AluOp.mult)
                            nc.vector.tensor_tensor(
                                dcx[:, fc, :], dcx[:, fc, :], t2[:, fc, :],
                                AluOp.add)
                            nc.vector.tensor_tensor(
                                t2[:, fc, :], ws[2], pt[2][:, fc, :],
                                AluOp.mult)
                            nc.vector.tensor_tensor(
                                dcx[:, fc, :], dcx[:, fc, :], t2[:, fc, :],
                                AluOp.add)
                        rh = slice((ch % 2) * RTA, (ch % 2) * RTA + RTA)
                        nc.gpsimd.dma_start(dec[ch // 2, :, 0:8, rh], dcx)

                    def emit_l1_group(i, ch):
                        p_i = pP.tile([128, 8, RTA], mm_dtype,
                                      tag=f"p{i}", name=f"p{i}")
                        xt = xts[ch]
                        for fc in range(8):
                            ps = psA.tile([128, RTA], F32, tag="mm",
                                          name="ps1")
                            for kc in range(4):
                                nc.tensor.matmul(
                                    ps, w_sb[i][:, fc, kc, :],
                                    xt[:, 4 * i + kc, :],
                                    start=(kc == 0), stop=(kc == 3))
                            nc.scalar.activation(
                                p_i[:, fc, :], ps, Act.Relu,
                                bias=b_sb[i][:, fc:fc + 1])
                        return p_i

                    def emit_l2_att(ch, pt):
                        last = plast.tile([128, 8, RTA], mm_dtype, tag="last")
                        for fc in range(8):
                            ps = psA.tile([128, RTA], F32, tag="mm",
                                          name="ps2")
                            for i in range(3):
                                for kc in range(8):
                                    nc.tensor.matmul(
                                        ps, wh_sb[:, fc, 8 * i + kc, :],
                                        pt[i][:, kc, :],
                                        start=(i == 0 and kc == 0),
                                        stop=(i == 2 and kc == 7))
                            nc.scalar.activation(last[:, fc, :], ps, Act.Relu,
                                                 bias=bh_sb[:, fc:fc + 1])
                        rh = slice((ch % 2) * RTA, (ch % 2) * RTA + RTA)
                        nc.gpsimd.dma_start(dec[ch // 2, :, 8:16, rh], last)
                        tmps = []
                        for i in range(3):
                            tmp = pscr.tile([128, 8, RTA], mm_dtype,
                                            tag="scr", name=f"tmp{i}")
                            nc.vector.tensor_tensor(tmp, last, pt[i],
                                                    AluOp.mult)
                            tmps.append(tmp)
                        carry[ch] = (pt, last, tmps)

                    # --- warm-up: first 3 chunks run L1 in (i, chunk) order
                    # so compute reuses each weight tensor across 3 chunks
                    # the moment it lands (the weight stream can't keep up
                    # with a single chunk's consumption rate at t=0) ---
                    WARM = 3
                    ptc = {ch: [] for ch in range(WARM)}
                    for i in range(3):
                        for ch in range(WARM):
                            ptc[ch].append(emit_l1_group(i, ch))
                    for ch in range(WARM):
                        xts.pop(ch)
                    load_xt(3)
                    load_xt(4)
                    for ch in range(WARM):
                        emit_l2_att(ch, ptc.pop(ch))
                        if ch == 0:
                            # wd1 head-start columns: virgin SBUF, issued on
                            # the gpsimd ring after dec(0) so the transfer
                            # stays clear of the startup-critical window
                            for fc in range(WD1H):
                                nc.gpsimd.dma_start(wd1h[:, fc],
                                                    wd1.ap()[fc])
                        if ch >= 1:
                            emit_attention(ch - 1)

                    # --- steady state ---
                    for ch in range(WARM, NCHA):
                        pt = [emit_l1_group(i, ch) for i in range(3)]
                        if ch + 2 < NCHA:
                            load_xt(ch + 2)
                        emit_attention(ch - 1)
                        emit_l2_att(ch, pt)

                    psA_cm.__exit__(None, None, None)
                    # open psF right where psA's banks freed, so mm1 doesn't
                    # wait for psC (live through softmax(7)); psG isn't
                    # needed until mm2 (~55us later) so it can open normally
                    psF_cm = tc.tile_pool(name="psF", bufs=3, space="PSUM")
                    psF = psF_cm.__enter__()
                    emit_attention(NCHA - 1)

                # ================= PASS B =================
                with (
                    tc.tile_pool(name="pwB", bufs=1) as pwB,
                    tc.tile_pool(name="po1", bufs=2) as po1,
                    tc.tile_pool(name="pev", bufs=3) as pev,
                    tc.tile_pool(name="psG", bufs=3, space="PSUM") as psG,
                ):
                    bd1_sb = pwB.tile([128, 16], F32, tag="bd1")
                    nc.sync.dma_start(bd1_sb, bd1.ap())
                    bd2_sb = pwB.tile([128, 8], F32, tag="bd2")
                    nc.sync.dma_start(bd2_sb, bd2.ap())

                    load_dc(0)
                    load_dc(1)
                    # pass-B weights on the sync queue: it drains early in
                    # pass A, so these stream in as soon as their aliased
                    # SBUF (pass-A tiles) is released
                    wd1_sb = pwB.tile([128, 16 - WD1H, 16, 128], mm_dtype,
                                      tag="wd1")
                    wd2_sb = pwB.tile([128, 8, 16, 128], mm_dtype, tag="wd2")
                    for fc in range(WD1H, 16):
                        for kh in range(2):
                            ks = slice(8 * kh, 8 * kh + 8)
                            nc.sync.dma_start(wd1_sb[:, fc - WD1H, ks],
                                              wd1.ap()[fc, :, ks])
                    for fc in range(8):
                        nc.sync.dma_start(wd2_sb[:, fc], wd2.ap()[fc])

                    for ch in range(NCHB):
                        rs = slice(ch * RTB, (ch + 1) * RTB)
                        dc = dcs.pop(ch)
                        o1 = po1.tile([128, 16, RTB], mm_dtype, tag="o1")
                        for fc in range(16):
                            w1src = (wd1h[:, fc] if fc < WD1H
                                     else wd1_sb[:, fc - WD1H])
                            ps = psF.tile([128, RTB], F32, tag="f1")
                            for kc in range(16):
                                nc.tensor.matmul(ps, w1src[:, kc, :],
                                                 dc[:, kc, :],
                                                 start=(kc == 0),
                                                 stop=(kc == 15))
                            nc.scalar.activation(o1[:, fc, :], ps, Act.Relu,
                                                 bias=bd1_sb[:, fc:fc + 1])
                        # dc(ch) fully consumed; recycle its buffer
                        if ch + 2 < NCHB:
                            load_dc(ch + 2)
                        for oc in range(8):
                            ow = 128 if oc < 7 else OUT - 7 * 128
                            ps = psG.tile([128, RTB], F32, tag="f2")
                            for kc in range(16):
                                nc.tensor.matmul(ps, wd2_sb[:, oc, kc, :],
                                                 o1[:, kc, :],
                                                 start=(kc == 0),
                                                 stop=(kc == 15))
                            ev = pev.tile([128, RTB], F32, tag="ev")
                            nc.vector.tensor_scalar_add(ev, ps,
                                                        bd2_sb[:, oc:oc + 1])
                            for rh in range(4):
                                q = RTB // 4
                                rr = slice(ch * RTB + rh * q,
                                           ch * RTB + (rh + 1) * q)
                                cc = slice(rh * q, (rh + 1) * q)
                                nc.sync.dma_start(
                                    outD.ap()[oc * 128:oc * 128 + ow, rr],
                                    ev[:ow, cc])
                # LIFO: psF (opened before psG/pwB pools) closes last
                psF_cm.__exit__(None, None, None)

    nc.finalize()
    return nc


def _wpack(W, FC, KC, np_dt):
    """[F_out, K_in] -> [FC, 128p, KC, 128f] (pad F_out up to FC*128)."""
    W = np.asarray(W, np.float32)
    F, K = W.shape
    if F < FC * 128:
        W = np.pad(W, ((0, FC * 128 - F), (0, 0)))
    W4 = W.reshape(FC, 128, KC, 128)          # [fc, f, kc, p]
    return np.ascontiguousarray(W4.transpose(0, 3, 2, 1)).astype(np_dt)


def _bmat(b, cc):
    """[F] -> [128, cc] so column c holds b[c*128:(c+1)*128]."""
    b = np.asarray(b, np.float32)
    if b.shape[0] < cc * 128:
        b = np.pad(b, (0, cc * 128 - b.shape[0]))
    return np.ascontiguousarray(b.reshape(cc, 128).T)


def _prep_inputs(np_dt, tube, w1_W, w1_b, w2_W, w2_b, w3_W, w3_b, wh_W, wh_b,
                 wd1_W, wd1_b, wd2_W, wd2_b):
    shared = {
        "w1": _wpack(w1_W, 8, 4, np_dt), "w2": _wpack(w2_W, 8, 4, np_dt),
        "w3": _wpack(w3_W, 8, 4, np_dt), "wh": _wpack(wh_W, 8, 24, np_dt),
        "wd1": _wpack(wd1_W, 16, 16, np_dt), "wd2": _wpack(wd2_W, 8, 16, np_dt),
        "b1": _bmat(w1_b, 8), "b2": _bmat(w2_b, 8), "b3": _bmat(w3_b, 8),
        "bh": _bmat(wh_b, 8), "bd1": _bmat(wd1_b, 16), "bd2": _bmat(wd2_b, 8),
    }
    tube = np.asarray(tube, np.float32)
    in_maps = []
    for c in range(N_CORES):
        t = tube[c * ROWS:(c + 1) * ROWS]                # [ROWS, 1536]
        xc = np.ascontiguousarray(
            t.reshape(NCHA, RTA, 12, 128).transpose(0, 3, 2, 1)
        ).astype(np_dt)                                  # [NCHA, 128, 12, RTA]
        in_maps.append({"xT": xc, **shared})
    return in_maps


_NC_CACHE = {}


def run(inputs, mm_dtype=BF16, trace=False):
    key = (mm_dtype,)
    if key not in _NC_CACHE:
        _NC_CACHE[key] = build_nc(mm_dtype)
    nc = _NC_CACHE[key]
    np_dt = ml_dtypes.bfloat16 if mm_dtype == BF16 else np.float32
    in_maps = _prep_inputs(np_dt, **inputs)
    res = run_bass_kernel_spmd(nc, in_maps, list(range(N_CORES)), trace=trace)
    out = np.empty((B, OUT), np.float32)
    for c in range(N_CORES):
        out[c * ROWS:(c + 1) * ROWS] = res.results[c]["out"].T
    return out, res


def kernel(**inputs) -> np.ndarray:
    out, _ = run(inputs)
    return out
